# revision 24
# baseline (speedup 1.0000x reference)
"""Trainium2 Bass kernel for nn_Loss_dict_50646254354805 (NeRF-style loss).

Self-contained: accepts FULL inputs, shards across 8 NeuronCores (rays for
the per-ray losses, samples for the hash loss), runs one SPMD Bass module,
host-sums the 8 partial scalars.

Inter-loss: merged-domain algorithm. Queries (prop_sdist) and blur events
(render_sdist +- pw) are reduced to 16-bit fixed-point sort keys with the
kind tag in the 2 LSBs, bitonic-merged per ray block (u16 min/max runs at
the DVE 2x 16-bit rate), and the blurred-density CDF is rebuilt over the
merged grid with prefix scans exactly like the reference's cumsum structure.
Exact fp32 positions are re-attached by per-partition local_scatter; key
quantization (6.3e-5) only perturbs interval assignment at coincidences and
is ~1e-6 on the loss. Counts/flags/index math runs in fp16 (2x/4x DVE
modes), u16-half interleave/deinterleave runs on the DMA engines, scatters
and the hash/distortion/rgb losses run on Pool, activations on Act. Both
levels share one slab of SBUF scratch (level 1 uses sliced views).
"""
import numpy as np

import concourse.bass as bass
import concourse.bass_isa as bass_isa
import concourse.mybir as mybir
import concourse.tile as tile
from concourse import bacc
from concourse.bass_utils import run_bass_kernel_spmd

dt = mybir.dt
Alu = mybir.AluOpType
AX = mybir.AxisListType
AF = mybir.ActivationFunctionType
P = 128

# problem constants
PULSE = (0.01, 0.005)
W_RGB, W_INTER, W_DIST, W_HASH = 1.0, 1.0, 0.01, 0.1
NUM_SEGMENTS = 65536
R, N = 4096, 48
M = R * N
N_CORES = 8
RPC = R // N_CORES            # rays per core (512)
NBLK = RPC // P               # ray tiles per core (4)
MPC = M // N_CORES            # hash samples per core (24576)
HALO = 64                     # hash run halo
HROW = MPC // P               # hash samples per partition (192)
HCOLS = HROW + HALO + 1       # loaded cols per partition (257)
HSLICE = HALO + MPC + HALO    # per-core hash slice length (24704)

# u16 fixed-point keys: key = (trunc((v + OFS) * SC4) & ~3) | tag
OFS = 0.02
SC4 = 63488.0

# per-level geometry
LVL = {0: dict(X=257, n2=512, LW=360, QWS=258),
       1: dict(X=97, n2=256, LW=200, QWS=98)}
X0, NL0, NQ0 = 257, NBLK * 360, NBLK * 258


def _ts_int(eng, out, in0, imm1, op0, imm2=None, op1=None):
    """tensor_scalar with int32 immediates (for bitwise/compare ops)."""
    ins_ = [eng.lower_ap(in0), mybir.ImmediateValue(dtype=dt.int32, value=int(imm1))]
    kw = dict(op0=op0)
    if imm2 is not None:
        ins_.append(mybir.ImmediateValue(dtype=dt.int32, value=int(imm2)))
        kw["op1"] = op1
    return eng.add_instruction(mybir.InstTensorScalarPtr(
        name=eng.bass.get_next_instruction_name(),
        ins=ins_, outs=[eng.lower_ap(out)], **kw))


def _blk(ap, n2):
    """[P, NBLK*n2] AP -> [P, NBLK, n2] view."""
    return ap.rearrange("p (b n) -> p b n", b=NBLK)


def _lo16(ap_f32):
    """fp32 AP -> strided u16 view of low halves."""
    return ap_f32.bitcast(dt.uint16).rearrange("p (n two) -> p n two", two=2)[:, :, 0]


def _hi16(ap_f32):
    return ap_f32.bitcast(dt.uint16).rearrange("p (n two) -> p n two", two=2)[:, :, 1]


def _lo16b(ap_f32, n):
    """fp32 [P, NBLK*n] AP -> [P, NBLK, n] view of low u16 halves."""
    return ap_f32.bitcast(dt.uint16).rearrange(
        "p (b n two) -> p b n two", b=NBLK, two=2)[:, :, :, 0]


def _hi16b(ap_f32, n):
    return ap_f32.bitcast(dt.uint16).rearrange(
        "p (b n two) -> p b n two", b=NBLK, two=2)[:, :, :, 1]


def _merge_u16(eng, cur_ap, nxt_ap, width, descending, pad_init=None):
    """Windowed ping-pong bitonic merge over [P, NBLK*width] u16 APs.

    pad_init: boolean [width] marking 0xFFFF pad slots. Pads move
    deterministically (pad loses min, wins max), so per stage only the
    contiguous hull of pairs touching >=1 real needs compare ops; pairs
    outside are pad-vs-pad and their slots are never read again. Output
    real slots form the prefix; trailing slots may hold garbage.
    """
    import numpy as np
    if pad_init is None:
        pad_init = np.zeros(width, bool)
    pad = pad_init.copy()
    cur, nxt = cur_ap, nxt_ap
    d = width // 2
    while d >= 1:
        i = np.arange(width).reshape(-1, 2 * d)[:, :d].reshape(-1)
        lo_pad, hi_pad = pad[i], pad[i + d]
        touch = np.where(~(lo_pad & hi_pad))[0]
        a, b = int(touch[0]), int(touch[-1]) + 1
        c4 = cur.rearrange("p (r q td) -> p r q td", r=NBLK, td=2 * d)
        n4 = nxt.rearrange("p (r q td) -> p r q td", r=NBLK, td=2 * d)
        ranges = []
        q0, r0 = divmod(a, d)
        q1, r1 = divmod(b, d)
        if r0:
            ranges.append((q0, q0 + 1, r0, d if q1 > q0 else r1))
            q0 += 1
        if q1 > q0:
            ranges.append((q0, q1, 0, d))
        if r1 and q1 >= q0:
            ranges.append((q1, q1 + 1, 0, r1))
        for (ba, bb, oa, ob) in ranges:
            lo_in = c4[:, :, ba:bb, oa:ob]
            hi_in = c4[:, :, ba:bb, d + oa:d + ob]
            if descending:
                eng.tensor_tensor(n4[:, :, ba:bb, oa:ob], lo_in, hi_in,
                                  Alu.max)
                eng.tensor_tensor(n4[:, :, ba:bb, d + oa:d + ob], lo_in,
                                  hi_in, Alu.min)
            else:
                eng.tensor_tensor(n4[:, :, ba:bb, oa:ob], lo_in, hi_in,
                                  Alu.min)
                eng.tensor_tensor(n4[:, :, ba:bb, d + oa:d + ob], lo_in,
                                  hi_in, Alu.max)
        npad = pad.copy()
        if descending:
            npad[i], npad[i + d] = lo_pad | hi_pad, lo_pad & hi_pad
        else:
            npad[i], npad[i + d] = lo_pad & hi_pad, lo_pad | hi_pad
        pad = npad
        cur, nxt = nxt, cur
        d //= 2
    return cur


def _emit_level(nc, tc, sl, lvl, s_sh, radio, x_t, pwt, inter_acc, cst):
    """Inter-loss pipeline for one prop level.

    Scatters use interleaved doubled indices (2i, 2i+1) over u16 views of
    fp32 tiles, so scattered fp32 values land assembled — no u16-half
    split/recombine stages. All scatters are per ray-block to stay inside
    the GPSIMD local-RAM limit."""
    L = LVL[lvl]
    X, n2, LW, QWS = L["X"], L["n2"], L["LW"], L["QWS"]
    NL = NBLK * LW
    NQ = NBLK * QWS
    pw = PULSE[lvl]
    V = nc.vector
    G = nc.gpsimd

    iotaG2 = cst[f"iotaG2_l{lvl}"][:]      # int16, block-local doubled: 2*i
    iotaP1G2 = cst[f"iotaP1G2_l{lvl}"][:]  # fp16, block-local: 2*i + 2
    mask16 = cst[f"mask16_l{lvl}"][:]
    mask32 = cst[f"mask32_l{lvl}"][:]

    def blkL(ap):
        return ap.rearrange("p (b n) -> p b n", b=NBLK)

    def S(name, w):            # slab view, width w
        return sl[name][:][:, 0:w]

    def u16v(ap_f32, w2):      # fp32 AP -> contiguous u16 pair view
        return ap_f32.bitcast(dt.uint16)[:, 0:w2]

    # ---------- exact shifted event values (fp32) ----------
    emsh = S("emsh", NBLK * 49)
    V.tensor_scalar(emsh, s_sh[:], -pw, None, Alu.add)
    epsh = S("epsh", NBLK * 49)
    V.tensor_scalar(epsh, s_sh[:], pw, None, Alu.add)

    # 1/(pw + 1e-5), needed only in the tail but hoisted off the critical path
    NW = NBLK * (X - 1)
    dinv = S("dinv", NW)
    nc.scalar.activation(dinv, pwt[:], AF.Copy, bias=1e-5)
    V.reciprocal(dinv, dinv)

    # ---------- u16 keys (written straight into the merge arrays) ----------
    b1a = S("b1a", NBLK * 128)
    b1b = S("b1b", NBLK * 128)
    b1a3 = _blk(b1a, 128)
    k16 = sl["k32"][:].bitcast(dt.uint16)[:, 0:NBLK * 49]
    V.memset(b1a3[:, :, 49:79], 0xFFFF)
    V.tensor_scalar(k16, emsh, OFS, SC4, Alu.add, Alu.mult)
    _ts_int(V, b1a3[:, :, 0:49], k16, 0xFFFC, Alu.bitwise_and, 1, Alu.bitwise_or)
    V.tensor_scalar(k16, epsh, OFS, SC4, Alu.add, Alu.mult)
    _ts_int(V, b1a3[:, :, 79:128][:, :, ::-1], k16, 0xFFFC, Alu.bitwise_and,
            3, Alu.bitwise_or)
    b1 = _merge_u16(V, b1a, b1b, 128, descending=True)

    B0 = S("B0", NBLK * n2)
    B1 = S("B1", NBLK * n2)
    B03 = _blk(B0, n2)
    G.memset(B03[:, :, X:n2 - 128], 0xFFFF)
    kq16 = sl["kq32"][:].bitcast(dt.uint16)[:, 0:NBLK * X]
    V.tensor_scalar(kq16, x_t[:], OFS, SC4, Alu.add, Alu.mult)
    _ts_int(V, B03[:, :, 0:X], kq16, 0xFFFC, Alu.bitwise_and)
    V.tensor_copy(B03[:, :, n2 - 128:n2], _blk(b1, 128))
    _bpad = np.zeros(n2, bool)
    _bpad[X:n2 - 98] = True
    SM = _merge_u16(V, B0, B1, n2, descending=False, pad_init=_bpad)
    mS = _blk(SM, n2)[:, :, 0:LW]      # compact real+pad prefix (u16 keys)
    V.memset(_blk(SM, n2)[:, :, X + 98:LW], 0xFFFF)

    # ---------- tags / flags (fp16) ----------
    tag16 = S("tag16", NL)
    _ts_int(V, tag16, mS, 3, Alu.bitwise_and)
    em = S("f16a", NL)
    _ts_int(V, em, tag16, 1, Alu.is_equal)
    ep = S("f16b", NL)
    _ts_int(V, ep, tag16, 3, Alu.is_equal)
    ev = S("f16c", NL)
    _ts_int(V, ev, tag16, 1, Alu.is_ge)
    qf = S("f16d", NL)
    _ts_int(V, qf, tag16, 0, Alu.is_equal)

    # ---------- counts ----------
    Cm = S("f16e", NL)
    V.tensor_tensor_scan(Cm, mask16, em, 0.0, Alu.mult, Alu.add)
    C = S("f16c", NL)                  # ev dead after this scan
    V.tensor_tensor_scan(C, mask16, ev, 0.0, Alu.mult, Alu.add)

    # ---------- masked ordinals -> int16 scatter indices ----------
    tm = S("f16f", NL)
    idxm = S("idxm", NL)
    V.tensor_tensor(tm, Cm, em, Alu.mult)
    V.tensor_scalar(idxm, tm, 1.0, None, Alu.subtract)
    pos_m = S("pos_m", NBLK * 64)      # doubled block-local slot of j-th em
    pos_p = S("pos_p", NBLK * 64)
    for b in range(NBLK):
        G.local_scatter(pos_m[:, b * 64:(b + 1) * 64],
                        iotaG2[:, b * LW:(b + 1) * LW],
                        idxm[:, b * LW:(b + 1) * LW], channels=P,
                        num_elems=64, num_idxs=LW)
    cep = S("f16a", NL)                # em dead after tm
    V.tensor_tensor(cep, C, Cm, Alu.subtract)
    V.tensor_tensor(tm, cep, ep, Alu.mult)
    V.tensor_scalar(idxm, tm, 1.0, None, Alu.subtract)
    for b in range(NBLK):
        G.local_scatter(pos_p[:, b * 64:(b + 1) * 64],
                        iotaG2[:, b * LW:(b + 1) * LW],
                        idxm[:, b * LW:(b + 1) * LW], channels=P,
                        num_elems=64, num_idxs=LW)
    tmq = S("f16f", NL)                # tmq = q_ordinal + 1 at query slots
    V.tensor_tensor(tmq, iotaP1G2, C, Alu.subtract)
    V.tensor_tensor(tmq, tmq, qf, Alu.mult)
    idxqL = S("idxm", NL)              # local q ordinal (qpos scatter)
    V.tensor_scalar(idxqL, tmq, 1.0, None, Alu.subtract)
    qpos = S("qpos", NQ)               # doubled block-local slot of k-th query
    for b in range(NBLK):
        G.local_scatter(qpos[:, b * QWS:(b + 1) * QWS],
                        iotaG2[:, b * LW:(b + 1) * LW],
                        idxqL[:, b * LW:(b + 1) * LW], channels=P,
                        num_elems=QWS, num_idxs=LW)
    idx2q = S("idx2q", 2 * NL)         # interleaved (2q, 2q+1)
    i2q3 = idx2q.rearrange("p (n two) -> p n two", two=2)
    G.tensor_scalar(i2q3[:, :, 0], tmq, 2.0, 2.0, Alu.mult, Alu.subtract)
    G.tensor_scalar(i2q3[:, :, 1], tmq, 2.0, 1.0, Alu.mult, Alu.subtract)

    # ---------- radio scatter (fp32 via interleaved doubled idx) ----------
    tgt2 = S("tgt2", NBLK * 256)       # interleaved (2t, 2t+1) event targets
    t2v = tgt2.rearrange("p (b n two) -> p b n two", b=NBLK, two=2)
    G.memset(t2v[:, :, 98:128, :], -1)
    pm3 = _blk(pos_m, 64)
    pp3 = _blk(pos_p, 64)
    G.tensor_copy(t2v[:, :, 0:49, 0], pm3[:, :, 0:49])
    _ts_int(G, t2v[:, :, 0:49, 1], pm3[:, :, 0:49], 1, Alu.add)
    G.tensor_copy(t2v[:, :, 49:98, 0], pp3[:, :, 0:49])
    _ts_int(G, t2v[:, :, 49:98, 1], pp3[:, :, 0:49], 1, Alu.add)

    radcat = S("radcat", NBLK * 128)
    r3 = _blk(radcat, 128)
    G.memset(r3[:, :, 98:128], 0.0)
    G.tensor_copy(r3[:, :, 0:49], _blk(radio[:], 49))
    G.tensor_scalar(r3[:, :, 49:98], _blk(radio[:], 49), -1.0, None, Alu.mult)
    radio_m = S("F_A", NL)
    for b in range(NBLK):
        G.local_scatter(u16v(radio_m, 2 * NL)[:, b * 2 * LW:(b + 1) * 2 * LW],
                        u16v(radcat, 2 * NBLK * 128)[:, b * 256:(b + 1) * 256],
                        tgt2[:, b * 256:(b + 1) * 256], channels=P,
                        num_elems=2 * LW, num_idxs=256)

    # ---------- slope scan ----------
    g = S("F_B", NL)
    V.tensor_tensor_scan(g, mask32, radio_m, 0.0, Alu.mult, Alu.add)

    # ---------- merged exact values via combined scatter ----------
    dat = S("dat", NL)                 # [emsh | epsh | x | pad] per block
    d3 = blkL(dat)
    G.memset(d3[:, :, 98 + X:LW], 0.0)
    nc.scalar.activation(d3[:, :, 0:49], _blk(emsh, 49), AF.Copy)
    nc.scalar.activation(d3[:, :, 49:98], _blk(epsh, 49), AF.Copy)
    nc.scalar.activation(d3[:, :, 98:98 + X], _blk(x_t[:], X), AF.Copy)
    vidx2 = S("vidx2", 2 * NL)
    vi4 = vidx2.rearrange("p (b n two) -> p b n two", b=NBLK, two=2)
    G.memset(vi4[:, :, 98 + X:LW, :], -1)
    G.tensor_copy(vidx2.bitcast(dt.int32).rearrange(
        "p (b n) -> p b n", b=NBLK)[:, :, 0:98],
        tgt2.bitcast(dt.int32).rearrange("p (b n) -> p b n", b=NBLK)[:, :, 0:98])
    q3 = _blk(qpos, QWS)
    G.tensor_copy(vi4[:, :, 98:98 + X, 0], q3[:, :, 0:X])
    _ts_int(G, vi4[:, :, 98:98 + X, 1], q3[:, :, 0:X], 1, Alu.add)
    v = S("F_C", NL)
    for b in range(NBLK):
        G.local_scatter(u16v(v, 2 * NL)[:, b * 2 * LW:(b + 1) * 2 * LW],
                        u16v(dat, 2 * NL)[:, b * 2 * LW:(b + 1) * 2 * LW],
                        vidx2[:, b * 2 * LW:(b + 1) * 2 * LW], channels=P,
                        num_elems=2 * LW, num_idxs=2 * LW)

    # ---------- density reconstruction ----------
    v3 = blkL(v)
    dv = S("F_D", NL)
    dv3 = blkL(dv)
    G.memset(dv3[:, :, 0:1], 0.0)
    G.tensor_tensor(dv3[:, :, 1:LW], v3[:, :, 1:LW], v3[:, :, 0:LW - 1],
                    Alu.subtract)
    wg = S("F_A", NL)                  # radio_m dead after g scan
    wg3 = blkL(wg)
    G.memset(wg3[:, :, 0:1], 0.0)
    G.tensor_tensor(wg3[:, :, 1:LW], dv3[:, :, 1:LW],
                    blkL(g)[:, :, 0:LW - 1], Alu.mult)
    w = S("F_C", NL)                   # v dead after dv
    V.tensor_tensor_scan(w, mask32, wg, 0.0, Alu.mult, Alu.add)
    wc = S("F_A", NL)                  # wg dead after w scan
    nc.scalar.activation(wc, w, AF.Relu, scale=0.5)
    scr = S("F_B", NL)                 # g dead after wg
    scr3 = blkL(scr)
    wc3 = blkL(wc)
    G.memset(scr3[:, :, 0:1], 0.0)
    V.tensor_tensor(scr3[:, :, 1:LW], wc3[:, :, 1:LW], wc3[:, :, 0:LW - 1],
                    Alu.add)
    area = S("F_C", NL)                # w dead after wc
    V.tensor_tensor(area, scr, dv, Alu.mult)
    cdf = S("F_D", NL)                 # dv dead after area
    V.tensor_tensor_scan(cdf, mask32, area, 0.0, Alu.mult, Alu.add)

    # ---------- compact cdf at query slots (fp32 direct) ----------
    cqf = sl["kq32"][:].bitcast(dt.float32)[:, 0:NQ]
    for b in range(NBLK):
        G.local_scatter(u16v(cqf, 2 * NQ)[:, b * 2 * QWS:(b + 1) * 2 * QWS],
                        u16v(cdf, 2 * NL)[:, b * 2 * LW:(b + 1) * 2 * LW],
                        idx2q[:, b * 2 * LW:(b + 1) * 2 * LW], channels=P,
                        num_elems=2 * QWS, num_idxs=2 * LW)

    # ---------- loss tail ----------
    cqf3 = _blk(cqf, QWS)
    ws = S("F_A", NW)                  # wc dead after scr
    V.tensor_tensor(_blk(ws, X - 1), cqf3[:, :, 1:X], cqf3[:, :, 0:X - 1],
                    Alu.subtract)
    V.tensor_tensor(ws, ws, pwt[:], Alu.subtract)
    rsl = S("F_B", NW)                 # scr dead after area
    nc.scalar.activation(rsl, ws, AF.Relu)
    G.tensor_tensor(rsl, rsl, dinv, Alu.mult)
    scrt = S("F_C", NW)
    G.tensor_tensor(scrt, ws, rsl, Alu.mult)
    V.tensor_reduce(inter_acc[:], _blk(scrt, X - 1), AX.XY, Alu.add)
    V.tensor_scalar(inter_acc[:], inter_acc[:], 1.0 / (R * (X - 1)), None,
                    Alu.mult)


def build_module():
    nc = bacc.Bacc("TRN2", target_bir_lowering=False, debug=False,
                   enable_asserts=False, num_devices=N_CORES)
    aps = {}

    def din(name, shape, dtype=dt.float32):
        aps[name] = nc.dram_tensor(name, shape, dtype, kind="ExternalInput").ap()
    din("pd", [RPC, 3]); din("gt", [RPC, 3])
    din("sd", [RPC, 49]); din("rw", [RPC, 48])
    din("ps0", [RPC, 257]); din("pw0", [RPC, 256])
    din("ps1", [RPC, 97]); din("pw1", [RPC, 96])
    din("hi0", [HSLICE], dt.int32); din("he0", [HSLICE * 2])
    din("hi1", [HSLICE], dt.int32); din("he1", [HSLICE * 2])
    out_ap = nc.dram_tensor("out", [1, 1], dt.float32, kind="ExternalOutput").ap()

    with tile.TileContext(nc) as tc:
        _emit(nc, tc, aps, out_ap)
    nc.compile()
    return nc


def _emit(nc, tc, aps, out_ap):
    import contextlib
    V = nc.vector
    G = nc.gpsimd
    with contextlib.ExitStack() as ctx:
        # ---------- generated constants ----------
        cpool = ctx.enter_context(tc.tile_pool(name="consts", bufs=1))
        cst = {}
        accs = {}
        for name in ("rgb", "inter", "inter1", "p1", "p2", "hash"):
            a = cpool.tile([P, 1], dt.float32, tag=f"acc_{name}")
            accs[name] = a
            V.memset(a[:], 0.0)

        # ---------- inputs ----------
        ipool = ctx.enter_context(tc.tile_pool(name="inputs", bufs=1))
        s_sh = ipool.tile([P, NBLK * 49], dt.float32, tag="s_sh")
        nc.sync.dma_start(_blk(s_sh[:], 49),
                          aps["sd"].rearrange("(b p) x -> p b x", p=P))
        rw_sh = ipool.tile([P, NBLK * 48], dt.float32, tag="rw_sh")
        nc.sync.dma_start(_blk(rw_sh[:], 48),
                          aps["rw"].rearrange("(b p) x -> p b x", p=P))
        xts, pwts = {}, {}
        for lvl, L in LVL.items():
            X = L["X"]
            xt = ipool.tile([P, NBLK * X], dt.float32, tag=f"xt{lvl}")
            nc.sync.dma_start(_blk(xt[:], X),
                              aps[f"ps{lvl}"].rearrange("(b p) x -> p b x", p=P))
            xts[lvl] = xt
            pwt = ipool.tile([P, NBLK * (X - 1)], dt.float32, tag=f"pwt{lvl}")
            nc.scalar.dma_start(_blk(pwt[:], X - 1),
                                aps[f"pw{lvl}"].rearrange("(b p) x -> p b x", p=P))
            pwts[lvl] = pwt

        # iotaG consts (needed first by mid-level scatters)
        for lvl, L in LVL.items():
            LW = L["LW"]
            NL = NBLK * LW
            t = cpool.tile([P, NL], dt.int16, tag=f"iotaG2_l{lvl}",
                           name=f"iotaG2_l{lvl}")
            G.iota(t[:], [[0, NBLK], [2, LW]], base=0, channel_multiplier=0)
            cst[f"iotaG2_l{lvl}"] = t

        # ---------- shared prep: wnorm, radio, mrad (Pool) ----------
        spool = ctx.enter_context(tc.tile_pool(name="shared", bufs=1))
        s3 = _blk(s_sh[:], 49)
        ds = spool.tile([P, NBLK * 48], dt.float32, tag="ds")
        G.tensor_tensor(_blk(ds[:], 48), s3[:, :, 1:49], s3[:, :, 0:48],
                        Alu.subtract)
        dse = spool.tile([P, NBLK * 48], dt.float32, tag="dse")
        G.tensor_scalar(dse[:], ds[:], 1e-8, None, Alu.add)
        wnorm = spool.tile([P, NBLK * 48], dt.float32, tag="wnorm")
        V.reciprocal(dse[:], dse[:])
        G.tensor_tensor(wnorm[:], rw_sh[:], dse[:], Alu.mult)
        wnp = spool.tile([P, NBLK * 50], dt.float32, tag="wnp")
        G.memset(wnp[:], 0.0)
        G.tensor_copy(_blk(wnp[:], 50)[:, :, 1:49], _blk(wnorm[:], 48))
        diff = spool.tile([P, NBLK * 49], dt.float32, tag="diff")
        wnp3 = _blk(wnp[:], 50)
        G.tensor_tensor(_blk(diff[:], 49), wnp3[:, :, 1:50], wnp3[:, :, 0:49],
                        Alu.subtract)
        radios = {}
        for lvl in (0, 1):
            rt = spool.tile([P, NBLK * 49], dt.float32, tag=f"radio{lvl}")
            G.tensor_scalar(rt[:], diff[:], 1.0 / (2 * PULSE[lvl]), None,
                            Alu.mult)
            radios[lvl] = rt

        # remaining consts: masks + iotaP1G (needed mid-level)
        iP1i = cpool.tile([P, NL0], dt.int16, tag="iP1i")
        for lvl, L in LVL.items():
            LW, QWS, NL = L["LW"], L["QWS"], NBLK * L["LW"]
            m16 = cpool.tile([P, NL], dt.float16, tag=f"mask16_l{lvl}",
                             name=f"mask16_l{lvl}")
            G.memset(m16[:], 1.0)
            G.memset(_blk(m16[:], LW)[:, :, 0:1], 0.0)
            cst[f"mask16_l{lvl}"] = m16
            m32 = cpool.tile([P, NL], dt.float32, tag=f"mask32_l{lvl}",
                             name=f"mask32_l{lvl}")
            G.memset(m32[:], 1.0)
            G.memset(_blk(m32[:], LW)[:, :, 0:1], 0.0)
            cst[f"mask32_l{lvl}"] = m32
            G.iota(iP1i[:][:, 0:NL], [[0, NBLK], [1, LW]], base=1,
                   channel_multiplier=0)
            tf = cpool.tile([P, NL], dt.float16, tag=f"iotaP1G2_l{lvl}",
                            name=f"iotaP1G2_l{lvl}")
            nc.scalar.activation(tf[:], iP1i[:][:, 0:NL], AF.Copy)
            cst[f"iotaP1G2_l{lvl}"] = tf
        mask48 = cpool.tile([P, NBLK * 48], dt.float32, tag="mask48")
        G.memset(mask48[:], 1.0)
        G.memset(_blk(mask48[:], 48)[:, :, 0:1], 0.0)
        ones_h = cpool.tile([P, HCOLS], dt.float32, tag="ones_h")
        G.memset(ones_h[:], 1.0)

        # ---------- per-level slabs ----------
        lpool = ctx.enter_context(tc.tile_pool(name="levels", bufs=1))
        slabs = {0: {}, 1: {}}
        for lvl, L in LVL.items():
            sl = slabs[lvl]
            NL = NBLK * L["LW"]
            NQ = NBLK * L["QWS"]
            n2 = L["n2"]

            def slab(name, width, dtype, lvl=lvl, sl=sl):
                nm = f"{name}_l{lvl}"
                sl[name] = lpool.tile([P, width], dtype, tag=nm, name=nm)
            slab("emsh", NBLK * 49, dt.float32)
            slab("epsh", NBLK * 49, dt.float32)
            slab("k32", NBLK * 49, dt.int32)
            slab("kq32", NQ, dt.int32)
            slab("b1a", NBLK * 128, dt.uint16)
            slab("b1b", NBLK * 128, dt.uint16)
            slab("B0", NBLK * n2, dt.uint16)
            slab("B1", NBLK * n2, dt.uint16)
            slab("tag16", NL, dt.uint16)
            for nm in ("f16a", "f16b", "f16c", "f16d", "f16e", "f16f"):
                slab(nm, NL, dt.float16)
            slab("idxm", NL, dt.int16)
            slab("idx2q", 2 * NL, dt.int16)
            slab("vidx2", 2 * NL, dt.int16)
            slab("pos_m", NBLK * 64, dt.int16)
            slab("pos_p", NBLK * 64, dt.int16)
            slab("qpos", NQ, dt.int16)
            slab("tgt2", NBLK * 256, dt.int16)
            slab("radcat", NBLK * 128, dt.float32)
            slab("dat", NL, dt.float32)
            for nm in ("F_A", "F_B", "F_C", "F_D"):
                slab(nm, NL, dt.float32)
            slab("dinv", NBLK * (L["X"] - 1), dt.float32)

        for lvl in (0, 1):
            _emit_level(nc, tc, slabs[lvl], lvl, s_sh, radios[lvl],
                        xts[lvl], pwts[lvl],
                        accs["inter" if lvl == 0 else "inter1"], cst)

        # ---------- distortion loss (Pool) ----------
        with tc.tile_pool(name="dist", bufs=1) as pool:
            mid = pool.tile([P, NBLK * 48], dt.float32, tag="mid")
            G.tensor_tensor(_blk(mid[:], 48), s3[:, :, 1:49],
                            s3[:, :, 0:48], Alu.add)
            G.tensor_scalar(mid[:], mid[:], 0.5, None, Alu.mult)
            wm = pool.tile([P, NBLK * 48], dt.float32, tag="wm")
            G.tensor_tensor(wm[:], rw_sh[:], mid[:], Alu.mult)
            Cin = pool.tile([P, NBLK * 48], dt.float32, tag="Cin")
            V.tensor_tensor_scan(Cin[:], mask48[:], rw_sh[:], 0.0,
                                 Alu.mult, Alu.add)
            Sin = pool.tile([P, NBLK * 48], dt.float32, tag="Sin")
            V.tensor_tensor_scan(Sin[:], mask48[:], wm[:], 0.0,
                                 Alu.mult, Alu.add)
            A = pool.tile([P, NBLK * 47], dt.float32, tag="A47")
            m3 = _blk(mid[:], 48)
            c3 = _blk(Cin[:], 48)
            sw3 = _blk(Sin[:], 48)
            rw3 = _blk(rw_sh[:], 48)
            A3 = _blk(A[:], 47)
            G.tensor_tensor(A3, m3[:, :, 1:48], c3[:, :, 0:47], Alu.mult)
            G.tensor_tensor(A3, A3, sw3[:, :, 0:47], Alu.subtract)
            G.tensor_tensor(A3, A3, rw3[:, :, 1:48], Alu.mult)
            V.tensor_reduce(accs["p1"][:], A3, AX.XY, Alu.add)
            t2 = pool.tile([P, NBLK * 48], dt.float32, tag="t2d")
            G.tensor_tensor(t2[:], rw_sh[:], rw_sh[:], Alu.mult)
            G.tensor_tensor(t2[:], t2[:], ds[:], Alu.mult)
            V.tensor_reduce(accs["p2"][:], _blk(t2[:], 48), AX.XY, Alu.add)

        # ---------- rgb (Pool) ----------
        with tc.tile_pool(name="rgb", bufs=1) as pool:
            pdt = pool.tile([P, NBLK * 3], dt.float32, tag="pdt")
            gtt = pool.tile([P, NBLK * 3], dt.float32, tag="gtt")
            nc.sync.dma_start(_blk(pdt[:], 3),
                              aps["pd"].rearrange("(b p) c -> p b c", p=P))
            nc.sync.dma_start(_blk(gtt[:], 3),
                              aps["gt"].rearrange("(b p) c -> p b c", p=P))
            d = pool.tile([P, NBLK * 3], dt.float32, tag="rgbd")
            G.tensor_tensor(d[:], pdt[:], gtt[:], Alu.subtract)
            G.tensor_tensor(d[:], d[:], d[:], Alu.mult)
            V.tensor_reduce(accs["rgb"][:], d[:], AX.X, Alu.add)

        # ---------- hash (Pool) ----------
        for lvl in (0, 1):
            with tc.tile_pool(name=f"hash{lvl}", bufs=1) as pool:
                idx = pool.tile([P, HCOLS], dt.int32, tag="hidx")
                src = aps[f"hi{lvl}"]
                nc.sync.dma_start(idx[:], bass.AP(tensor=src.tensor,
                                                  offset=src.offset,
                                                  ap=[[HROW, P], [1, HCOLS]]))
                emb = pool.tile([P, HCOLS * 2], dt.float32, tag="hemb")
                esrc = aps[f"he{lvl}"]
                nc.scalar.dma_start(emb[:], bass.AP(tensor=esrc.tensor,
                                                    offset=esrc.offset,
                                                    ap=[[HROW * 2, P],
                                                        [1, HCOLS * 2]]))
                sq = pool.tile([P, HCOLS * 2], dt.float32, tag="hsq")
                G.tensor_tensor(sq[:], emb[:], emb[:], Alu.mult)
                wv = pool.tile([P, HCOLS], dt.float32, tag="hw")
                sq3 = sq[:].rearrange("p (n two) -> p n two", two=2)
                G.tensor_tensor(wv[:], sq3[:, :, 0], sq3[:, :, 1], Alu.add)
                eq = pool.tile([P, HCOLS], dt.float32, tag="heq")
                G.memset(eq[:, 0:1], 0.0)
                dq = pool.tile([P, HCOLS], dt.int32, tag="hdq")
                G.tensor_tensor(dq[:, 1:HCOLS], idx[:, 1:HCOLS],
                                idx[:, 0:HCOLS - 1], Alu.subtract)
                _ts_int(G, eq[:, 1:HCOLS], dq[:, 1:HCOLS], 0, Alu.is_equal)
                S = pool.tile([P, HCOLS], dt.float32, tag="hS")
                V.tensor_tensor_scan(S[:], eq[:], wv[:], 0.0,
                                     Alu.mult, Alu.add)
                cc = pool.tile([P, HCOLS], dt.float32, tag="hcc")
                V.tensor_tensor_scan(cc[:], eq[:], ones_h[:], 0.0,
                                     Alu.mult, Alu.add)
                ratio = pool.tile([P, HCOLS], dt.float32, tag="hr")
                V.reciprocal(cc[:], cc[:])
                G.tensor_tensor(ratio[:], S[:], cc[:], Alu.mult)
                me = pool.tile([P, HCOLS], dt.float32, tag="hme")
                G.tensor_scalar(me[:, 0:HCOLS - 1], eq[:, 1:HCOLS], -1.0,
                                1.0, Alu.mult, Alu.add)
                G.tensor_tensor(ratio[:, HALO:HALO + HROW],
                                ratio[:, HALO:HALO + HROW],
                                me[:, HALO:HALO + HROW], Alu.mult)
                part = pool.tile([P, 1], dt.float32, tag="hpart")
                V.tensor_reduce(part[:], ratio[:, HALO:HALO + HROW],
                                AX.X, Alu.add)
                if lvl == 0:
                    V.tensor_copy(accs["hash"][:], part[:])
                else:
                    V.tensor_tensor(accs["hash"][:], accs["hash"][:],
                                    part[:], Alu.add)

        # ---------- combine + output ----------
        with tc.tile_pool(name="fin", bufs=1) as pool:
            tot = pool.tile([P, 1], dt.float32, tag="tot")
            V.tensor_scalar(tot[:], accs["rgb"][:], W_RGB / (R * 3), None,
                            Alu.mult)
            V.scalar_tensor_tensor(tot[:], accs["inter"][:], W_INTER,
                                   tot[:], Alu.mult, Alu.add)
            V.scalar_tensor_tensor(tot[:], accs["inter1"][:], W_INTER,
                                   tot[:], Alu.mult, Alu.add)
            V.scalar_tensor_tensor(tot[:], accs["p1"][:], W_DIST * 2.0 / R,
                                   tot[:], Alu.mult, Alu.add)
            V.scalar_tensor_tensor(tot[:], accs["p2"][:],
                                   W_DIST / (3.0 * R), tot[:],
                                   Alu.mult, Alu.add)
            V.scalar_tensor_tensor(tot[:], accs["hash"][:],
                                   W_HASH / (NUM_SEGMENTS * 2.0), tot[:],
                                   Alu.mult, Alu.add)
            res = pool.tile([P, 1], dt.float32, tag="res")
            G.partition_all_reduce(res[:], tot[:], channels=P,
                                   reduce_op=bass_isa.ReduceOp.add)
            nc.sync.dma_start(out_ap, res[0:1, :])


# ---------------- host side ----------------
_module_cache = {}


def _get_module():
    if "nc" not in _module_cache:
        _module_cache["nc"] = build_module()
    return _module_cache["nc"]


def shard_inputs(inputs):
    """Full inputs -> list of 8 per-core in_maps."""
    f32 = np.float32
    pd = np.ascontiguousarray(inputs["pd_rgbs"], f32)
    gt = np.ascontiguousarray(inputs["gt_rgbs"], f32)
    sd = np.ascontiguousarray(inputs["render_sdist"], f32)
    rw = np.ascontiguousarray(inputs["render_weights"], f32)
    ps0 = np.ascontiguousarray(inputs["prop_sdist_0"], f32)
    pw0 = np.ascontiguousarray(inputs["prop_weights_0"], f32)
    ps1 = np.ascontiguousarray(inputs["prop_sdist_1"], f32)
    pw1 = np.ascontiguousarray(inputs["prop_weights_1"], f32)
    hashes = {}
    for lvl in (0, 1):
        idx = np.asarray(inputs[f"enc_idx_{lvl}"]).astype(np.int32)
        emb = np.ascontiguousarray(inputs[f"enc_embds_{lvl}"], f32)
        idx_pad = np.full(M + 2 * HALO, -1, np.int32)
        idx_pad[HALO:HALO + M] = idx
        emb_pad = np.zeros((M + 2 * HALO, 2), f32)
        emb_pad[HALO:HALO + M] = emb
        hashes[lvl] = (idx_pad, emb_pad)

    in_maps = []
    for c in range(N_CORES):
        r0 = c * RPC
        lo = c * MPC
        im = {
            "pd": pd[r0:r0 + RPC], "gt": gt[r0:r0 + RPC],
            "sd": sd[r0:r0 + RPC], "rw": rw[r0:r0 + RPC],
            "ps0": ps0[r0:r0 + RPC], "pw0": pw0[r0:r0 + RPC],
            "ps1": ps1[r0:r0 + RPC], "pw1": pw1[r0:r0 + RPC],
        }
        for lvl in (0, 1):
            idx_pad, emb_pad = hashes[lvl]
            im[f"hi{lvl}"] = np.ascontiguousarray(idx_pad[lo:lo + HSLICE])
            im[f"he{lvl}"] = np.ascontiguousarray(
                emb_pad[lo:lo + HSLICE].reshape(-1))
        in_maps.append(im)
    return in_maps


def kernel(**inputs) -> np.ndarray:
    nc = _get_module()
    in_maps = shard_inputs(inputs)
    res = run_bass_kernel_spmd(nc, in_maps, core_ids=list(range(N_CORES)))
    total = np.float64(0.0)
    for r in res.results:
        total += np.float64(r["out"][0, 0])
    return np.float32(total)


# revision 25
# speedup vs baseline: 1.0032x; 1.0032x over previous
"""Trainium2 Bass kernel for nn_Loss_dict_50646254354805 (NeRF-style loss).

Self-contained: accepts FULL inputs, shards across 8 NeuronCores (rays for
the per-ray losses, samples for the hash loss), runs one SPMD Bass module,
host-sums the 8 partial scalars.

Inter-loss: merged-domain algorithm. Queries (prop_sdist) and blur events
(render_sdist +- pw) are reduced to 16-bit fixed-point sort keys with the
kind tag in the 2 LSBs, bitonic-merged per ray block (u16 min/max runs at
the DVE 2x 16-bit rate), and the blurred-density CDF is rebuilt over the
merged grid with prefix scans exactly like the reference's cumsum structure.
Exact fp32 positions are re-attached by per-partition local_scatter; key
quantization (6.3e-5) only perturbs interval assignment at coincidences and
is ~1e-6 on the loss. Counts/flags/index math runs in fp16 (2x/4x DVE
modes), u16-half interleave/deinterleave runs on the DMA engines, scatters
and the hash/distortion/rgb losses run on Pool, activations on Act. Both
levels share one slab of SBUF scratch (level 1 uses sliced views).
"""
import numpy as np

import concourse.bass as bass
import concourse.bass_isa as bass_isa
import concourse.mybir as mybir
import concourse.tile as tile
from concourse import bacc
from concourse.bass_utils import run_bass_kernel_spmd

dt = mybir.dt
Alu = mybir.AluOpType
AX = mybir.AxisListType
AF = mybir.ActivationFunctionType
P = 128

# problem constants
PULSE = (0.01, 0.005)
W_RGB, W_INTER, W_DIST, W_HASH = 1.0, 1.0, 0.01, 0.1
NUM_SEGMENTS = 65536
R, N = 4096, 48
M = R * N
N_CORES = 8
RPC = R // N_CORES            # rays per core (512)
NBLK = RPC // P               # ray tiles per core (4)
MPC = M // N_CORES            # hash samples per core (24576)
HALO = 64                     # hash run halo
HROW = MPC // P               # hash samples per partition (192)
HCOLS = HROW + HALO + 1       # loaded cols per partition (257)
HSLICE = HALO + MPC + HALO    # per-core hash slice length (24704)

# u16 fixed-point keys: key = (trunc((v + OFS) * SC4) & ~3) | tag
OFS = 0.02
SC4 = 63488.0

# per-level geometry
LVL = {0: dict(X=257, n2=512, LW=360, QWS=258),
       1: dict(X=97, n2=256, LW=200, QWS=98)}
X0, NL0, NQ0 = 257, NBLK * 360, NBLK * 258


def _ts_int(eng, out, in0, imm1, op0, imm2=None, op1=None):
    """tensor_scalar with int32 immediates (for bitwise/compare ops)."""
    ins_ = [eng.lower_ap(in0), mybir.ImmediateValue(dtype=dt.int32, value=int(imm1))]
    kw = dict(op0=op0)
    if imm2 is not None:
        ins_.append(mybir.ImmediateValue(dtype=dt.int32, value=int(imm2)))
        kw["op1"] = op1
    return eng.add_instruction(mybir.InstTensorScalarPtr(
        name=eng.bass.get_next_instruction_name(),
        ins=ins_, outs=[eng.lower_ap(out)], **kw))


def _blk(ap, n2):
    """[P, NBLK*n2] AP -> [P, NBLK, n2] view."""
    return ap.rearrange("p (b n) -> p b n", b=NBLK)


def _lo16(ap_f32):
    """fp32 AP -> strided u16 view of low halves."""
    return ap_f32.bitcast(dt.uint16).rearrange("p (n two) -> p n two", two=2)[:, :, 0]


def _hi16(ap_f32):
    return ap_f32.bitcast(dt.uint16).rearrange("p (n two) -> p n two", two=2)[:, :, 1]


def _lo16b(ap_f32, n):
    """fp32 [P, NBLK*n] AP -> [P, NBLK, n] view of low u16 halves."""
    return ap_f32.bitcast(dt.uint16).rearrange(
        "p (b n two) -> p b n two", b=NBLK, two=2)[:, :, :, 0]


def _hi16b(ap_f32, n):
    return ap_f32.bitcast(dt.uint16).rearrange(
        "p (b n two) -> p b n two", b=NBLK, two=2)[:, :, :, 1]


def _merge_u16(eng, cur_ap, nxt_ap, width, descending, pad_init=None):
    """Windowed ping-pong bitonic merge over [P, NBLK*width] u16 APs.

    pad_init: boolean [width] marking 0xFFFF pad slots. Pads move
    deterministically (pad loses min, wins max), so per stage only the
    contiguous hull of pairs touching >=1 real needs compare ops; pairs
    outside are pad-vs-pad and their slots are never read again. Output
    real slots form the prefix; trailing slots may hold garbage.
    """
    import numpy as np
    if pad_init is None:
        pad_init = np.zeros(width, bool)
    pad = pad_init.copy()
    cur, nxt = cur_ap, nxt_ap
    d = width // 2
    while d >= 1:
        i = np.arange(width).reshape(-1, 2 * d)[:, :d].reshape(-1)
        lo_pad, hi_pad = pad[i], pad[i + d]
        touch = np.where(~(lo_pad & hi_pad))[0]
        a, b = int(touch[0]), int(touch[-1]) + 1
        c4 = cur.rearrange("p (r q td) -> p r q td", r=NBLK, td=2 * d)
        n4 = nxt.rearrange("p (r q td) -> p r q td", r=NBLK, td=2 * d)
        ranges = []
        q0, r0 = divmod(a, d)
        q1, r1 = divmod(b, d)
        if r0:
            ranges.append((q0, q0 + 1, r0, d if q1 > q0 else r1))
            q0 += 1
        if q1 > q0:
            ranges.append((q0, q1, 0, d))
        if r1 and q1 >= q0:
            ranges.append((q1, q1 + 1, 0, r1))
        for (ba, bb, oa, ob) in ranges:
            lo_in = c4[:, :, ba:bb, oa:ob]
            hi_in = c4[:, :, ba:bb, d + oa:d + ob]
            if descending:
                eng.tensor_tensor(n4[:, :, ba:bb, oa:ob], lo_in, hi_in,
                                  Alu.max)
                eng.tensor_tensor(n4[:, :, ba:bb, d + oa:d + ob], lo_in,
                                  hi_in, Alu.min)
            else:
                eng.tensor_tensor(n4[:, :, ba:bb, oa:ob], lo_in, hi_in,
                                  Alu.min)
                eng.tensor_tensor(n4[:, :, ba:bb, d + oa:d + ob], lo_in,
                                  hi_in, Alu.max)
        npad = pad.copy()
        if descending:
            npad[i], npad[i + d] = lo_pad | hi_pad, lo_pad & hi_pad
        else:
            npad[i], npad[i + d] = lo_pad & hi_pad, lo_pad | hi_pad
        pad = npad
        cur, nxt = nxt, cur
        d //= 2
    return cur


def _emit_level(nc, tc, sl, lvl, s_sh, radio, x_t, pwt, inter_acc, cst):
    """Inter-loss pipeline for one prop level.

    Scatters use interleaved doubled indices (2i, 2i+1) over u16 views of
    fp32 tiles, so scattered fp32 values land assembled — no u16-half
    split/recombine stages. All scatters are per ray-block to stay inside
    the GPSIMD local-RAM limit."""
    L = LVL[lvl]
    X, n2, LW, QWS = L["X"], L["n2"], L["LW"], L["QWS"]
    NL = NBLK * LW
    NQ = NBLK * QWS
    pw = PULSE[lvl]
    V = nc.vector
    G = nc.gpsimd

    iotaG2 = cst[f"iotaG2_l{lvl}"][:]      # int16, block-local doubled: 2*i
    iotaP1G2 = cst[f"iotaP1G2_l{lvl}"][:]  # fp16, block-local: 2*i + 2
    mask16 = cst[f"mask16_l{lvl}"][:]
    mask32 = cst[f"mask32_l{lvl}"][:]

    def blkL(ap):
        return ap.rearrange("p (b n) -> p b n", b=NBLK)

    def S(name, w):            # slab view, width w
        return sl[name][:][:, 0:w]

    def u16v(ap_f32, w2):      # fp32 AP -> contiguous u16 pair view
        return ap_f32.bitcast(dt.uint16)[:, 0:w2]

    # ---------- exact shifted event values (fp32) ----------
    emsh = S("emsh", NBLK * 49)
    V.tensor_scalar(emsh, s_sh[:], -pw, None, Alu.add)
    epsh = S("epsh", NBLK * 49)
    V.tensor_scalar(epsh, s_sh[:], pw, None, Alu.add)

    # 1/(pw + 1e-5), needed only in the tail but hoisted off the critical path
    NW = NBLK * (X - 1)
    dinv = S("dinv", NW)
    nc.scalar.activation(dinv, pwt[:], AF.Copy, bias=1e-5)
    V.reciprocal(dinv, dinv)

    # ---------- u16 keys (written straight into the merge arrays) ----------
    b1a = S("b1a", NBLK * 128)
    b1b = S("b1b", NBLK * 128)
    b1a3 = _blk(b1a, 128)
    k16 = sl["k32"][:].bitcast(dt.uint16)[:, 0:NBLK * 49]
    V.memset(b1a3[:, :, 49:79], 0xFFFF)
    V.tensor_scalar(k16, emsh, OFS, SC4, Alu.add, Alu.mult)
    _ts_int(V, b1a3[:, :, 0:49], k16, 0xFFFC, Alu.bitwise_and, 1, Alu.bitwise_or)
    V.tensor_scalar(k16, epsh, OFS, SC4, Alu.add, Alu.mult)
    _ts_int(V, b1a3[:, :, 79:128][:, :, ::-1], k16, 0xFFFC, Alu.bitwise_and,
            3, Alu.bitwise_or)
    b1 = _merge_u16(V, b1a, b1b, 128, descending=True)

    B0 = S("B0", NBLK * n2)
    B1 = S("B1", NBLK * n2)
    B03 = _blk(B0, n2)
    V.memset(B03[:, :, X:n2 - 128], 0xFFFF)
    kq16 = sl["kq32"][:].bitcast(dt.uint16)[:, 0:NBLK * X]
    V.tensor_scalar(kq16, x_t[:], OFS, SC4, Alu.add, Alu.mult)
    _ts_int(V, B03[:, :, 0:X], kq16, 0xFFFC, Alu.bitwise_and)
    V.tensor_copy(B03[:, :, n2 - 128:n2], _blk(b1, 128))
    _bpad = np.zeros(n2, bool)
    _bpad[X:n2 - 98] = True
    SM = _merge_u16(V, B0, B1, n2, descending=False, pad_init=_bpad)
    mS = _blk(SM, n2)[:, :, 0:LW]      # compact real+pad prefix (u16 keys)
    V.memset(_blk(SM, n2)[:, :, X + 98:LW], 0xFFFF)

    # ---------- tags / flags (fp16) ----------
    tag16 = S("tag16", NL)
    _ts_int(V, tag16, mS, 3, Alu.bitwise_and)
    em = S("f16a", NL)
    _ts_int(V, em, tag16, 1, Alu.is_equal)
    ep = S("f16b", NL)
    _ts_int(V, ep, tag16, 3, Alu.is_equal)
    ev = S("f16c", NL)
    _ts_int(V, ev, tag16, 1, Alu.is_ge)
    qf = S("f16d", NL)
    _ts_int(V, qf, tag16, 0, Alu.is_equal)

    # ---------- counts ----------
    Cm = S("f16e", NL)
    V.tensor_tensor_scan(Cm, mask16, em, 0.0, Alu.mult, Alu.add)
    C = S("f16c", NL)                  # ev dead after this scan
    V.tensor_tensor_scan(C, mask16, ev, 0.0, Alu.mult, Alu.add)

    # ---------- masked ordinals -> int16 scatter indices ----------
    tm = S("f16f", NL)
    idxm = S("idxm", NL)
    V.tensor_tensor(tm, Cm, em, Alu.mult)
    V.tensor_scalar(idxm, tm, 1.0, None, Alu.subtract)
    pos_m = S("pos_m", NBLK * 64)      # doubled block-local slot of j-th em
    pos_p = S("pos_p", NBLK * 64)
    for b in range(NBLK):
        G.local_scatter(pos_m[:, b * 64:(b + 1) * 64],
                        iotaG2[:, b * LW:(b + 1) * LW],
                        idxm[:, b * LW:(b + 1) * LW], channels=P,
                        num_elems=64, num_idxs=LW)
    cep = S("f16a", NL)                # em dead after tm
    V.tensor_tensor(cep, C, Cm, Alu.subtract)
    V.tensor_tensor(tm, cep, ep, Alu.mult)
    V.tensor_scalar(idxm, tm, 1.0, None, Alu.subtract)
    for b in range(NBLK):
        G.local_scatter(pos_p[:, b * 64:(b + 1) * 64],
                        iotaG2[:, b * LW:(b + 1) * LW],
                        idxm[:, b * LW:(b + 1) * LW], channels=P,
                        num_elems=64, num_idxs=LW)
    tmq = S("f16f", NL)                # tmq = q_ordinal + 1 at query slots
    V.tensor_tensor(tmq, iotaP1G2, C, Alu.subtract)
    V.tensor_tensor(tmq, tmq, qf, Alu.mult)
    idxqL = S("idxm", NL)              # local q ordinal (qpos scatter)
    V.tensor_scalar(idxqL, tmq, 1.0, None, Alu.subtract)
    qpos = S("qpos", NQ)               # doubled block-local slot of k-th query
    for b in range(NBLK):
        G.local_scatter(qpos[:, b * QWS:(b + 1) * QWS],
                        iotaG2[:, b * LW:(b + 1) * LW],
                        idxqL[:, b * LW:(b + 1) * LW], channels=P,
                        num_elems=QWS, num_idxs=LW)
    idx2q = S("idx2q", 2 * NL)         # interleaved (2q, 2q+1)
    i2q3 = idx2q.rearrange("p (n two) -> p n two", two=2)
    G.tensor_scalar(i2q3[:, :, 0], tmq, 2.0, 2.0, Alu.mult, Alu.subtract)
    G.tensor_scalar(i2q3[:, :, 1], tmq, 2.0, 1.0, Alu.mult, Alu.subtract)

    # ---------- radio scatter (fp32 via interleaved doubled idx) ----------
    tgt2 = S("tgt2", NBLK * 256)       # interleaved (2t, 2t+1) event targets
    t2v = tgt2.rearrange("p (b n two) -> p b n two", b=NBLK, two=2)
    G.memset(t2v[:, :, 98:128, :], -1)
    pm3 = _blk(pos_m, 64)
    pp3 = _blk(pos_p, 64)
    G.tensor_copy(t2v[:, :, 0:49, 0], pm3[:, :, 0:49])
    _ts_int(G, t2v[:, :, 0:49, 1], pm3[:, :, 0:49], 1, Alu.add)
    G.tensor_copy(t2v[:, :, 49:98, 0], pp3[:, :, 0:49])
    _ts_int(G, t2v[:, :, 49:98, 1], pp3[:, :, 0:49], 1, Alu.add)

    radcat = S("radcat", NBLK * 128)
    r3 = _blk(radcat, 128)
    G.memset(r3[:, :, 98:128], 0.0)
    G.tensor_copy(r3[:, :, 0:49], _blk(radio[:], 49))
    G.tensor_scalar(r3[:, :, 49:98], _blk(radio[:], 49), -1.0, None, Alu.mult)
    radio_m = S("F_A", NL)
    for b in range(NBLK):
        G.local_scatter(u16v(radio_m, 2 * NL)[:, b * 2 * LW:(b + 1) * 2 * LW],
                        u16v(radcat, 2 * NBLK * 128)[:, b * 256:(b + 1) * 256],
                        tgt2[:, b * 256:(b + 1) * 256], channels=P,
                        num_elems=2 * LW, num_idxs=256)

    # ---------- slope scan ----------
    g = S("F_B", NL)
    V.tensor_tensor_scan(g, mask32, radio_m, 0.0, Alu.mult, Alu.add)

    # ---------- merged exact values via combined scatter ----------
    dat = S("dat", NL)                 # [emsh | epsh | x | pad] per block
    d3 = blkL(dat)
    G.memset(d3[:, :, 98 + X:LW], 0.0)
    nc.scalar.activation(d3[:, :, 0:49], _blk(emsh, 49), AF.Copy)
    nc.scalar.activation(d3[:, :, 49:98], _blk(epsh, 49), AF.Copy)
    nc.scalar.activation(d3[:, :, 98:98 + X], _blk(x_t[:], X), AF.Copy)
    vidx2 = S("vidx2", 2 * NL)
    vi4 = vidx2.rearrange("p (b n two) -> p b n two", b=NBLK, two=2)
    G.memset(vi4[:, :, 98 + X:LW, :], -1)
    G.tensor_copy(vidx2.bitcast(dt.int32).rearrange(
        "p (b n) -> p b n", b=NBLK)[:, :, 0:98],
        tgt2.bitcast(dt.int32).rearrange("p (b n) -> p b n", b=NBLK)[:, :, 0:98])
    q3 = _blk(qpos, QWS)
    G.tensor_copy(vi4[:, :, 98:98 + X, 0], q3[:, :, 0:X])
    _ts_int(G, vi4[:, :, 98:98 + X, 1], q3[:, :, 0:X], 1, Alu.add)
    v = S("F_C", NL)
    for b in range(NBLK):
        G.local_scatter(u16v(v, 2 * NL)[:, b * 2 * LW:(b + 1) * 2 * LW],
                        u16v(dat, 2 * NL)[:, b * 2 * LW:(b + 1) * 2 * LW],
                        vidx2[:, b * 2 * LW:(b + 1) * 2 * LW], channels=P,
                        num_elems=2 * LW, num_idxs=2 * LW)

    # ---------- density reconstruction ----------
    v3 = blkL(v)
    dv = S("F_D", NL)
    dv3 = blkL(dv)
    G.memset(dv3[:, :, 0:1], 0.0)
    G.tensor_tensor(dv3[:, :, 1:LW], v3[:, :, 1:LW], v3[:, :, 0:LW - 1],
                    Alu.subtract)
    wg = S("F_A", NL)                  # radio_m dead after g scan
    wg3 = blkL(wg)
    G.memset(wg3[:, :, 0:1], 0.0)
    G.tensor_tensor(wg3[:, :, 1:LW], dv3[:, :, 1:LW],
                    blkL(g)[:, :, 0:LW - 1], Alu.mult)
    w = S("F_C", NL)                   # v dead after dv
    V.tensor_tensor_scan(w, mask32, wg, 0.0, Alu.mult, Alu.add)
    wc = S("F_A", NL)                  # wg dead after w scan
    nc.scalar.activation(wc, w, AF.Relu, scale=0.5)
    scr = S("F_B", NL)                 # g dead after wg
    scr3 = blkL(scr)
    wc3 = blkL(wc)
    G.memset(scr3[:, :, 0:1], 0.0)
    V.tensor_tensor(scr3[:, :, 1:LW], wc3[:, :, 1:LW], wc3[:, :, 0:LW - 1],
                    Alu.add)
    area = S("F_C", NL)                # w dead after wc
    V.tensor_tensor(area, scr, dv, Alu.mult)
    cdf = S("F_D", NL)                 # dv dead after area
    V.tensor_tensor_scan(cdf, mask32, area, 0.0, Alu.mult, Alu.add)

    # ---------- compact cdf at query slots (fp32 direct) ----------
    cqf = sl["kq32"][:].bitcast(dt.float32)[:, 0:NQ]
    for b in range(NBLK):
        G.local_scatter(u16v(cqf, 2 * NQ)[:, b * 2 * QWS:(b + 1) * 2 * QWS],
                        u16v(cdf, 2 * NL)[:, b * 2 * LW:(b + 1) * 2 * LW],
                        idx2q[:, b * 2 * LW:(b + 1) * 2 * LW], channels=P,
                        num_elems=2 * QWS, num_idxs=2 * LW)

    # ---------- loss tail ----------
    cqf3 = _blk(cqf, QWS)
    ws = S("F_A", NW)                  # wc dead after scr
    V.tensor_tensor(_blk(ws, X - 1), cqf3[:, :, 1:X], cqf3[:, :, 0:X - 1],
                    Alu.subtract)
    V.tensor_tensor(ws, ws, pwt[:], Alu.subtract)
    rsl = S("F_B", NW)                 # scr dead after area
    nc.scalar.activation(rsl, ws, AF.Relu)
    G.tensor_tensor(rsl, rsl, dinv, Alu.mult)
    scrt = S("F_C", NW)
    G.tensor_tensor(scrt, ws, rsl, Alu.mult)
    V.tensor_reduce(inter_acc[:], _blk(scrt, X - 1), AX.XY, Alu.add)
    V.tensor_scalar(inter_acc[:], inter_acc[:], 1.0 / (R * (X - 1)), None,
                    Alu.mult)


def build_module():
    nc = bacc.Bacc("TRN2", target_bir_lowering=False, debug=False,
                   enable_asserts=False, num_devices=N_CORES)
    aps = {}

    def din(name, shape, dtype=dt.float32):
        aps[name] = nc.dram_tensor(name, shape, dtype, kind="ExternalInput").ap()
    din("pd", [RPC, 3]); din("gt", [RPC, 3])
    din("sd", [RPC, 49]); din("rw", [RPC, 48])
    din("ps0", [RPC, 257]); din("pw0", [RPC, 256])
    din("ps1", [RPC, 97]); din("pw1", [RPC, 96])
    din("hi0", [HSLICE], dt.int32); din("he0", [HSLICE * 2])
    din("hi1", [HSLICE], dt.int32); din("he1", [HSLICE * 2])
    out_ap = nc.dram_tensor("out", [1, 1], dt.float32, kind="ExternalOutput").ap()

    with tile.TileContext(nc) as tc:
        _emit(nc, tc, aps, out_ap)
    nc.compile()
    return nc


def _emit(nc, tc, aps, out_ap):
    import contextlib
    V = nc.vector
    G = nc.gpsimd
    with contextlib.ExitStack() as ctx:
        # ---------- generated constants ----------
        cpool = ctx.enter_context(tc.tile_pool(name="consts", bufs=1))
        cst = {}
        accs = {}
        for name in ("rgb", "inter", "inter1", "p1", "p2", "hash"):
            a = cpool.tile([P, 1], dt.float32, tag=f"acc_{name}")
            accs[name] = a
            V.memset(a[:], 0.0)

        # ---------- inputs ----------
        ipool = ctx.enter_context(tc.tile_pool(name="inputs", bufs=1))
        s_sh = ipool.tile([P, NBLK * 49], dt.float32, tag="s_sh")
        nc.sync.dma_start(_blk(s_sh[:], 49),
                          aps["sd"].rearrange("(b p) x -> p b x", p=P))
        rw_sh = ipool.tile([P, NBLK * 48], dt.float32, tag="rw_sh")
        nc.sync.dma_start(_blk(rw_sh[:], 48),
                          aps["rw"].rearrange("(b p) x -> p b x", p=P))
        xts, pwts = {}, {}
        for lvl, L in LVL.items():
            X = L["X"]
            xt = ipool.tile([P, NBLK * X], dt.float32, tag=f"xt{lvl}")
            nc.sync.dma_start(_blk(xt[:], X),
                              aps[f"ps{lvl}"].rearrange("(b p) x -> p b x", p=P))
            xts[lvl] = xt
            pwt = ipool.tile([P, NBLK * (X - 1)], dt.float32, tag=f"pwt{lvl}")
            nc.scalar.dma_start(_blk(pwt[:], X - 1),
                                aps[f"pw{lvl}"].rearrange("(b p) x -> p b x", p=P))
            pwts[lvl] = pwt

        # iotaG consts (needed first by mid-level scatters)
        for lvl, L in LVL.items():
            LW = L["LW"]
            NL = NBLK * LW
            t = cpool.tile([P, NL], dt.int16, tag=f"iotaG2_l{lvl}",
                           name=f"iotaG2_l{lvl}")
            G.iota(t[:], [[0, NBLK], [2, LW]], base=0, channel_multiplier=0)
            cst[f"iotaG2_l{lvl}"] = t

        # ---------- shared prep: wnorm, radio, mrad (Pool) ----------
        spool = ctx.enter_context(tc.tile_pool(name="shared", bufs=1))
        s3 = _blk(s_sh[:], 49)
        ds = spool.tile([P, NBLK * 48], dt.float32, tag="ds")
        G.tensor_tensor(_blk(ds[:], 48), s3[:, :, 1:49], s3[:, :, 0:48],
                        Alu.subtract)
        dse = spool.tile([P, NBLK * 48], dt.float32, tag="dse")
        G.tensor_scalar(dse[:], ds[:], 1e-8, None, Alu.add)
        wnorm = spool.tile([P, NBLK * 48], dt.float32, tag="wnorm")
        V.reciprocal(dse[:], dse[:])
        G.tensor_tensor(wnorm[:], rw_sh[:], dse[:], Alu.mult)
        wnp = spool.tile([P, NBLK * 50], dt.float32, tag="wnp")
        G.memset(wnp[:], 0.0)
        G.tensor_copy(_blk(wnp[:], 50)[:, :, 1:49], _blk(wnorm[:], 48))
        diff = spool.tile([P, NBLK * 49], dt.float32, tag="diff")
        wnp3 = _blk(wnp[:], 50)
        G.tensor_tensor(_blk(diff[:], 49), wnp3[:, :, 1:50], wnp3[:, :, 0:49],
                        Alu.subtract)
        radios = {}
        for lvl in (0, 1):
            rt = spool.tile([P, NBLK * 49], dt.float32, tag=f"radio{lvl}")
            G.tensor_scalar(rt[:], diff[:], 1.0 / (2 * PULSE[lvl]), None,
                            Alu.mult)
            radios[lvl] = rt

        # remaining consts: masks + iotaP1G (needed mid-level)
        iP1i = cpool.tile([P, NL0], dt.int16, tag="iP1i")
        for lvl, L in LVL.items():
            LW, QWS, NL = L["LW"], L["QWS"], NBLK * L["LW"]
            m16 = cpool.tile([P, NL], dt.float16, tag=f"mask16_l{lvl}",
                             name=f"mask16_l{lvl}")
            G.memset(m16[:], 1.0)
            G.memset(_blk(m16[:], LW)[:, :, 0:1], 0.0)
            cst[f"mask16_l{lvl}"] = m16
            m32 = cpool.tile([P, NL], dt.float32, tag=f"mask32_l{lvl}",
                             name=f"mask32_l{lvl}")
            G.memset(m32[:], 1.0)
            G.memset(_blk(m32[:], LW)[:, :, 0:1], 0.0)
            cst[f"mask32_l{lvl}"] = m32
            G.iota(iP1i[:][:, 0:NL], [[0, NBLK], [1, LW]], base=1,
                   channel_multiplier=0)
            tf = cpool.tile([P, NL], dt.float16, tag=f"iotaP1G2_l{lvl}",
                            name=f"iotaP1G2_l{lvl}")
            nc.scalar.activation(tf[:], iP1i[:][:, 0:NL], AF.Copy)
            cst[f"iotaP1G2_l{lvl}"] = tf
        mask48 = cpool.tile([P, NBLK * 48], dt.float32, tag="mask48")
        G.memset(mask48[:], 1.0)
        G.memset(_blk(mask48[:], 48)[:, :, 0:1], 0.0)
        ones_h = cpool.tile([P, HCOLS], dt.float32, tag="ones_h")
        G.memset(ones_h[:], 1.0)

        # ---------- per-level slabs ----------
        lpool = ctx.enter_context(tc.tile_pool(name="levels", bufs=1))
        slabs = {0: {}, 1: {}}
        for lvl, L in LVL.items():
            sl = slabs[lvl]
            NL = NBLK * L["LW"]
            NQ = NBLK * L["QWS"]
            n2 = L["n2"]

            def slab(name, width, dtype, lvl=lvl, sl=sl):
                nm = f"{name}_l{lvl}"
                sl[name] = lpool.tile([P, width], dtype, tag=nm, name=nm)
            slab("emsh", NBLK * 49, dt.float32)
            slab("epsh", NBLK * 49, dt.float32)
            slab("k32", NBLK * 49, dt.int32)
            slab("kq32", NQ, dt.int32)
            slab("b1a", NBLK * 128, dt.uint16)
            slab("b1b", NBLK * 128, dt.uint16)
            slab("B0", NBLK * n2, dt.uint16)
            slab("B1", NBLK * n2, dt.uint16)
            slab("tag16", NL, dt.uint16)
            for nm in ("f16a", "f16b", "f16c", "f16d", "f16e", "f16f"):
                slab(nm, NL, dt.float16)
            slab("idxm", NL, dt.int16)
            slab("idx2q", 2 * NL, dt.int16)
            slab("vidx2", 2 * NL, dt.int16)
            slab("pos_m", NBLK * 64, dt.int16)
            slab("pos_p", NBLK * 64, dt.int16)
            slab("qpos", NQ, dt.int16)
            slab("tgt2", NBLK * 256, dt.int16)
            slab("radcat", NBLK * 128, dt.float32)
            slab("dat", NL, dt.float32)
            for nm in ("F_A", "F_B", "F_C", "F_D"):
                slab(nm, NL, dt.float32)
            slab("dinv", NBLK * (L["X"] - 1), dt.float32)

        for lvl in (0, 1):
            _emit_level(nc, tc, slabs[lvl], lvl, s_sh, radios[lvl],
                        xts[lvl], pwts[lvl],
                        accs["inter" if lvl == 0 else "inter1"], cst)

        # ---------- distortion loss (Pool) ----------
        with tc.tile_pool(name="dist", bufs=1) as pool:
            mid = pool.tile([P, NBLK * 48], dt.float32, tag="mid")
            G.tensor_tensor(_blk(mid[:], 48), s3[:, :, 1:49],
                            s3[:, :, 0:48], Alu.add)
            G.tensor_scalar(mid[:], mid[:], 0.5, None, Alu.mult)
            wm = pool.tile([P, NBLK * 48], dt.float32, tag="wm")
            G.tensor_tensor(wm[:], rw_sh[:], mid[:], Alu.mult)
            Cin = pool.tile([P, NBLK * 48], dt.float32, tag="Cin")
            V.tensor_tensor_scan(Cin[:], mask48[:], rw_sh[:], 0.0,
                                 Alu.mult, Alu.add)
            Sin = pool.tile([P, NBLK * 48], dt.float32, tag="Sin")
            V.tensor_tensor_scan(Sin[:], mask48[:], wm[:], 0.0,
                                 Alu.mult, Alu.add)
            A = pool.tile([P, NBLK * 47], dt.float32, tag="A47")
            m3 = _blk(mid[:], 48)
            c3 = _blk(Cin[:], 48)
            sw3 = _blk(Sin[:], 48)
            rw3 = _blk(rw_sh[:], 48)
            A3 = _blk(A[:], 47)
            G.tensor_tensor(A3, m3[:, :, 1:48], c3[:, :, 0:47], Alu.mult)
            G.tensor_tensor(A3, A3, sw3[:, :, 0:47], Alu.subtract)
            G.tensor_tensor(A3, A3, rw3[:, :, 1:48], Alu.mult)
            V.tensor_reduce(accs["p1"][:], A3, AX.XY, Alu.add)
            t2 = pool.tile([P, NBLK * 48], dt.float32, tag="t2d")
            G.tensor_tensor(t2[:], rw_sh[:], rw_sh[:], Alu.mult)
            G.tensor_tensor(t2[:], t2[:], ds[:], Alu.mult)
            V.tensor_reduce(accs["p2"][:], _blk(t2[:], 48), AX.XY, Alu.add)

        # ---------- rgb (Pool) ----------
        with tc.tile_pool(name="rgb", bufs=1) as pool:
            pdt = pool.tile([P, NBLK * 3], dt.float32, tag="pdt")
            gtt = pool.tile([P, NBLK * 3], dt.float32, tag="gtt")
            nc.sync.dma_start(_blk(pdt[:], 3),
                              aps["pd"].rearrange("(b p) c -> p b c", p=P))
            nc.sync.dma_start(_blk(gtt[:], 3),
                              aps["gt"].rearrange("(b p) c -> p b c", p=P))
            d = pool.tile([P, NBLK * 3], dt.float32, tag="rgbd")
            G.tensor_tensor(d[:], pdt[:], gtt[:], Alu.subtract)
            G.tensor_tensor(d[:], d[:], d[:], Alu.mult)
            V.tensor_reduce(accs["rgb"][:], d[:], AX.X, Alu.add)

        # ---------- hash (Pool) ----------
        for lvl in (0, 1):
            with tc.tile_pool(name=f"hash{lvl}", bufs=1) as pool:
                idx = pool.tile([P, HCOLS], dt.int32, tag="hidx")
                src = aps[f"hi{lvl}"]
                nc.sync.dma_start(idx[:], bass.AP(tensor=src.tensor,
                                                  offset=src.offset,
                                                  ap=[[HROW, P], [1, HCOLS]]))
                emb = pool.tile([P, HCOLS * 2], dt.float32, tag="hemb")
                esrc = aps[f"he{lvl}"]
                nc.scalar.dma_start(emb[:], bass.AP(tensor=esrc.tensor,
                                                    offset=esrc.offset,
                                                    ap=[[HROW * 2, P],
                                                        [1, HCOLS * 2]]))
                sq = pool.tile([P, HCOLS * 2], dt.float32, tag="hsq")
                G.tensor_tensor(sq[:], emb[:], emb[:], Alu.mult)
                wv = pool.tile([P, HCOLS], dt.float32, tag="hw")
                sq3 = sq[:].rearrange("p (n two) -> p n two", two=2)
                G.tensor_tensor(wv[:], sq3[:, :, 0], sq3[:, :, 1], Alu.add)
                eq = pool.tile([P, HCOLS], dt.float32, tag="heq")
                G.memset(eq[:, 0:1], 0.0)
                dq = pool.tile([P, HCOLS], dt.int32, tag="hdq")
                G.tensor_tensor(dq[:, 1:HCOLS], idx[:, 1:HCOLS],
                                idx[:, 0:HCOLS - 1], Alu.subtract)
                _ts_int(G, eq[:, 1:HCOLS], dq[:, 1:HCOLS], 0, Alu.is_equal)
                S = pool.tile([P, HCOLS], dt.float32, tag="hS")
                V.tensor_tensor_scan(S[:], eq[:], wv[:], 0.0,
                                     Alu.mult, Alu.add)
                cc = pool.tile([P, HCOLS], dt.float32, tag="hcc")
                V.tensor_tensor_scan(cc[:], eq[:], ones_h[:], 0.0,
                                     Alu.mult, Alu.add)
                ratio = pool.tile([P, HCOLS], dt.float32, tag="hr")
                V.reciprocal(cc[:], cc[:])
                G.tensor_tensor(ratio[:], S[:], cc[:], Alu.mult)
                me = pool.tile([P, HCOLS], dt.float32, tag="hme")
                G.tensor_scalar(me[:, 0:HCOLS - 1], eq[:, 1:HCOLS], -1.0,
                                1.0, Alu.mult, Alu.add)
                G.tensor_tensor(ratio[:, HALO:HALO + HROW],
                                ratio[:, HALO:HALO + HROW],
                                me[:, HALO:HALO + HROW], Alu.mult)
                part = pool.tile([P, 1], dt.float32, tag="hpart")
                V.tensor_reduce(part[:], ratio[:, HALO:HALO + HROW],
                                AX.X, Alu.add)
                if lvl == 0:
                    V.tensor_copy(accs["hash"][:], part[:])
                else:
                    V.tensor_tensor(accs["hash"][:], accs["hash"][:],
                                    part[:], Alu.add)

        # ---------- combine + output ----------
        with tc.tile_pool(name="fin", bufs=1) as pool:
            tot = pool.tile([P, 1], dt.float32, tag="tot")
            V.tensor_scalar(tot[:], accs["rgb"][:], W_RGB / (R * 3), None,
                            Alu.mult)
            V.scalar_tensor_tensor(tot[:], accs["inter"][:], W_INTER,
                                   tot[:], Alu.mult, Alu.add)
            V.scalar_tensor_tensor(tot[:], accs["inter1"][:], W_INTER,
                                   tot[:], Alu.mult, Alu.add)
            V.scalar_tensor_tensor(tot[:], accs["p1"][:], W_DIST * 2.0 / R,
                                   tot[:], Alu.mult, Alu.add)
            V.scalar_tensor_tensor(tot[:], accs["p2"][:],
                                   W_DIST / (3.0 * R), tot[:],
                                   Alu.mult, Alu.add)
            V.scalar_tensor_tensor(tot[:], accs["hash"][:],
                                   W_HASH / (NUM_SEGMENTS * 2.0), tot[:],
                                   Alu.mult, Alu.add)
            res = pool.tile([P, 1], dt.float32, tag="res")
            G.partition_all_reduce(res[:], tot[:], channels=P,
                                   reduce_op=bass_isa.ReduceOp.add)
            nc.sync.dma_start(out_ap, res[0:1, :])


# ---------------- host side ----------------
_module_cache = {}


def _get_module():
    if "nc" not in _module_cache:
        _module_cache["nc"] = build_module()
    return _module_cache["nc"]


def shard_inputs(inputs):
    """Full inputs -> list of 8 per-core in_maps."""
    f32 = np.float32
    pd = np.ascontiguousarray(inputs["pd_rgbs"], f32)
    gt = np.ascontiguousarray(inputs["gt_rgbs"], f32)
    sd = np.ascontiguousarray(inputs["render_sdist"], f32)
    rw = np.ascontiguousarray(inputs["render_weights"], f32)
    ps0 = np.ascontiguousarray(inputs["prop_sdist_0"], f32)
    pw0 = np.ascontiguousarray(inputs["prop_weights_0"], f32)
    ps1 = np.ascontiguousarray(inputs["prop_sdist_1"], f32)
    pw1 = np.ascontiguousarray(inputs["prop_weights_1"], f32)
    hashes = {}
    for lvl in (0, 1):
        idx = np.asarray(inputs[f"enc_idx_{lvl}"]).astype(np.int32)
        emb = np.ascontiguousarray(inputs[f"enc_embds_{lvl}"], f32)
        idx_pad = np.full(M + 2 * HALO, -1, np.int32)
        idx_pad[HALO:HALO + M] = idx
        emb_pad = np.zeros((M + 2 * HALO, 2), f32)
        emb_pad[HALO:HALO + M] = emb
        hashes[lvl] = (idx_pad, emb_pad)

    in_maps = []
    for c in range(N_CORES):
        r0 = c * RPC
        lo = c * MPC
        im = {
            "pd": pd[r0:r0 + RPC], "gt": gt[r0:r0 + RPC],
            "sd": sd[r0:r0 + RPC], "rw": rw[r0:r0 + RPC],
            "ps0": ps0[r0:r0 + RPC], "pw0": pw0[r0:r0 + RPC],
            "ps1": ps1[r0:r0 + RPC], "pw1": pw1[r0:r0 + RPC],
        }
        for lvl in (0, 1):
            idx_pad, emb_pad = hashes[lvl]
            im[f"hi{lvl}"] = np.ascontiguousarray(idx_pad[lo:lo + HSLICE])
            im[f"he{lvl}"] = np.ascontiguousarray(
                emb_pad[lo:lo + HSLICE].reshape(-1))
        in_maps.append(im)
    return in_maps


def kernel(**inputs) -> np.ndarray:
    nc = _get_module()
    in_maps = shard_inputs(inputs)
    res = run_bass_kernel_spmd(nc, in_maps, core_ids=list(range(N_CORES)))
    total = np.float64(0.0)
    for r in res.results:
        total += np.float64(r["out"][0, 0])
    return np.float32(total)


# revision 26
# speedup vs baseline: 1.0302x; 1.0269x over previous
"""Trainium2 Bass kernel for nn_Loss_dict_50646254354805 (NeRF-style loss).

Self-contained: accepts FULL inputs, shards across 8 NeuronCores (rays for
the per-ray losses, samples for the hash loss), runs one SPMD Bass module,
host-sums the 8 partial scalars.

Inter-loss: merged-domain algorithm. Queries (prop_sdist) and blur events
(render_sdist +- pw) are reduced to 16-bit fixed-point sort keys with the
kind tag in the 2 LSBs, bitonic-merged per ray block (u16 min/max runs at
the DVE 2x 16-bit rate), and the blurred-density CDF is rebuilt over the
merged grid with prefix scans exactly like the reference's cumsum structure.
Exact fp32 positions are re-attached by per-partition local_scatter; key
quantization (6.3e-5) only perturbs interval assignment at coincidences and
is ~1e-6 on the loss. Counts/flags/index math runs in fp16 (2x/4x DVE
modes), u16-half interleave/deinterleave runs on the DMA engines, scatters
and the hash/distortion/rgb losses run on Pool, activations on Act. Both
levels share one slab of SBUF scratch (level 1 uses sliced views).
"""
import numpy as np

import concourse.bass as bass
import concourse.bass_isa as bass_isa
import concourse.mybir as mybir
import concourse.tile as tile
from concourse import bacc
from concourse.bass_utils import run_bass_kernel_spmd

dt = mybir.dt
Alu = mybir.AluOpType
AX = mybir.AxisListType
AF = mybir.ActivationFunctionType
P = 128

# problem constants
PULSE = (0.01, 0.005)
W_RGB, W_INTER, W_DIST, W_HASH = 1.0, 1.0, 0.01, 0.1
NUM_SEGMENTS = 65536
R, N = 4096, 48
M = R * N
N_CORES = 8
RPC = R // N_CORES            # rays per core (512)
NBLK = RPC // P               # ray tiles per core (4)
MPC = M // N_CORES            # hash samples per core (24576)
HALO = 64                     # hash run halo
HROW = MPC // P               # hash samples per partition (192)
HCOLS = HROW + HALO + 1       # loaded cols per partition (257)
HSLICE = HALO + MPC + HALO    # per-core hash slice length (24704)

# u16 fixed-point keys: key = (trunc((v + OFS) * SC4) & ~3) | tag
OFS = 0.02
SC4 = 63488.0

# per-level geometry
LVL = {0: dict(X=257, n2=512, LW=360, QWS=258),
       1: dict(X=97, n2=256, LW=200, QWS=98)}
X0, NL0, NQ0 = 257, NBLK * 360, NBLK * 258


def _ts_int(eng, out, in0, imm1, op0, imm2=None, op1=None):
    """tensor_scalar with int32 immediates (for bitwise/compare ops)."""
    ins_ = [eng.lower_ap(in0), mybir.ImmediateValue(dtype=dt.int32, value=int(imm1))]
    kw = dict(op0=op0)
    if imm2 is not None:
        ins_.append(mybir.ImmediateValue(dtype=dt.int32, value=int(imm2)))
        kw["op1"] = op1
    return eng.add_instruction(mybir.InstTensorScalarPtr(
        name=eng.bass.get_next_instruction_name(),
        ins=ins_, outs=[eng.lower_ap(out)], **kw))


def _blk(ap, n2):
    """[P, NBLK*n2] AP -> [P, NBLK, n2] view."""
    return ap.rearrange("p (b n) -> p b n", b=NBLK)


def _lo16(ap_f32):
    """fp32 AP -> strided u16 view of low halves."""
    return ap_f32.bitcast(dt.uint16).rearrange("p (n two) -> p n two", two=2)[:, :, 0]


def _hi16(ap_f32):
    return ap_f32.bitcast(dt.uint16).rearrange("p (n two) -> p n two", two=2)[:, :, 1]


def _lo16b(ap_f32, n):
    """fp32 [P, NBLK*n] AP -> [P, NBLK, n] view of low u16 halves."""
    return ap_f32.bitcast(dt.uint16).rearrange(
        "p (b n two) -> p b n two", b=NBLK, two=2)[:, :, :, 0]


def _hi16b(ap_f32, n):
    return ap_f32.bitcast(dt.uint16).rearrange(
        "p (b n two) -> p b n two", b=NBLK, two=2)[:, :, :, 1]


def _merge_u16(eng, cur_ap, nxt_ap, width, descending, pad_init=None):
    """Windowed ping-pong bitonic merge over [P, NBLK*width] u16 APs.

    pad_init: boolean [width] marking 0xFFFF pad slots. Pads move
    deterministically (pad loses min, wins max), so per stage only the
    contiguous hull of pairs touching >=1 real needs compare ops; pairs
    outside are pad-vs-pad and their slots are never read again. Output
    real slots form the prefix; trailing slots may hold garbage.
    """
    import numpy as np
    if pad_init is None:
        pad_init = np.zeros(width, bool)
    pad = pad_init.copy()
    cur, nxt = cur_ap, nxt_ap
    d = width // 2
    while d >= 1:
        i = np.arange(width).reshape(-1, 2 * d)[:, :d].reshape(-1)
        lo_pad, hi_pad = pad[i], pad[i + d]
        touch = np.where(~(lo_pad & hi_pad))[0]
        a, b = int(touch[0]), int(touch[-1]) + 1
        c4 = cur.rearrange("p (r q td) -> p r q td", r=NBLK, td=2 * d)
        n4 = nxt.rearrange("p (r q td) -> p r q td", r=NBLK, td=2 * d)
        ranges = []
        q0, r0 = divmod(a, d)
        q1, r1 = divmod(b, d)
        if r0:
            ranges.append((q0, q0 + 1, r0, d if q1 > q0 else r1))
            q0 += 1
        if q1 > q0:
            ranges.append((q0, q1, 0, d))
        if r1 and q1 >= q0:
            ranges.append((q1, q1 + 1, 0, r1))
        for (ba, bb, oa, ob) in ranges:
            lo_in = c4[:, :, ba:bb, oa:ob]
            hi_in = c4[:, :, ba:bb, d + oa:d + ob]
            if descending:
                eng.tensor_tensor(n4[:, :, ba:bb, oa:ob], lo_in, hi_in,
                                  Alu.max)
                eng.tensor_tensor(n4[:, :, ba:bb, d + oa:d + ob], lo_in,
                                  hi_in, Alu.min)
            else:
                eng.tensor_tensor(n4[:, :, ba:bb, oa:ob], lo_in, hi_in,
                                  Alu.min)
                eng.tensor_tensor(n4[:, :, ba:bb, d + oa:d + ob], lo_in,
                                  hi_in, Alu.max)
        npad = pad.copy()
        if descending:
            npad[i], npad[i + d] = lo_pad | hi_pad, lo_pad & hi_pad
        else:
            npad[i], npad[i + d] = lo_pad & hi_pad, lo_pad | hi_pad
        pad = npad
        cur, nxt = nxt, cur
        d //= 2
    return cur


def _emit_level(nc, tc, sl, lvl, s_sh, radio, x_t, pwt, inter_acc, cst):
    """Inter-loss pipeline for one prop level.

    Scatters use interleaved doubled indices (2i, 2i+1) over u16 views of
    fp32 tiles, so scattered fp32 values land assembled — no u16-half
    split/recombine stages. All scatters are per ray-block to stay inside
    the GPSIMD local-RAM limit."""
    L = LVL[lvl]
    X, n2, LW, QWS = L["X"], L["n2"], L["LW"], L["QWS"]
    NL = NBLK * LW
    NQ = NBLK * QWS
    pw = PULSE[lvl]
    V = nc.vector
    G = nc.gpsimd

    iotaG2 = cst[f"iotaG2_l{lvl}"][:]      # int16, block-local doubled: 2*i
    iotaP1G2 = cst[f"iotaP1G2_l{lvl}"][:]  # fp16, block-local: 2*i + 2
    mask16 = cst[f"mask16_l{lvl}"][:]
    mask32 = cst[f"mask32_l{lvl}"][:]

    def blkL(ap):
        return ap.rearrange("p (b n) -> p b n", b=NBLK)

    def S(name, w):            # slab view, width w
        return sl[name][:][:, 0:w]

    def u16v(ap_f32, w2):      # fp32 AP -> contiguous u16 pair view
        return ap_f32.bitcast(dt.uint16)[:, 0:w2]

    # ---------- exact shifted event values (fp32) ----------
    emsh = S("emsh", NBLK * 49)
    V.tensor_scalar(emsh, s_sh[:], -pw, None, Alu.add)
    epsh = S("epsh", NBLK * 49)
    V.tensor_scalar(epsh, s_sh[:], pw, None, Alu.add)

    # 1/(pw + 1e-5), needed only in the tail but hoisted off the critical path
    NW = NBLK * (X - 1)
    dinv = S("dinv", NW)
    nc.scalar.activation(dinv, pwt[:], AF.Copy, bias=1e-5)
    V.reciprocal(dinv, dinv)

    # ---------- u16 keys (written straight into the merge arrays) ----------
    b1a = S("b1a", NBLK * 128)
    b1b = S("b1b", NBLK * 128)
    b1a3 = _blk(b1a, 128)
    k16 = sl["k32"][:].bitcast(dt.uint16)[:, 0:NBLK * 49]
    V.memset(b1a3[:, :, 49:79], 0xFFFF)
    V.tensor_scalar(k16, emsh, OFS, SC4, Alu.add, Alu.mult)
    _ts_int(V, b1a3[:, :, 0:49], k16, 0xFFFC, Alu.bitwise_and, 1, Alu.bitwise_or)
    V.tensor_scalar(k16, epsh, OFS, SC4, Alu.add, Alu.mult)
    _ts_int(V, b1a3[:, :, 79:128][:, :, ::-1], k16, 0xFFFC, Alu.bitwise_and,
            3, Alu.bitwise_or)
    b1 = _merge_u16(V, b1a, b1b, 128, descending=True)

    B0 = S("B0", NBLK * n2)
    B1 = S("B1", NBLK * n2)
    B03 = _blk(B0, n2)
    V.memset(B03[:, :, X:n2 - 128], 0xFFFF)
    kq16 = sl["kq32"][:].bitcast(dt.uint16)[:, 0:NBLK * X]
    V.tensor_scalar(kq16, x_t[:], OFS, SC4, Alu.add, Alu.mult)
    _ts_int(V, B03[:, :, 0:X], kq16, 0xFFFC, Alu.bitwise_and)
    V.tensor_copy(B03[:, :, n2 - 128:n2], _blk(b1, 128))
    _bpad = np.zeros(n2, bool)
    _bpad[X:n2 - 98] = True
    SM = _merge_u16(V, B0, B1, n2, descending=False, pad_init=_bpad)
    mS = _blk(SM, n2)[:, :, 0:LW]      # compact real+pad prefix (u16 keys)
    V.memset(_blk(SM, n2)[:, :, X + 98:LW], 0xFFFF)

    # ---------- tags / flags (fp16) ----------
    tag16 = S("tag16", NL)
    _ts_int(V, tag16, mS, 3, Alu.bitwise_and)
    em = S("f16a", NL)
    _ts_int(V, em, tag16, 1, Alu.is_equal)
    ep = S("f16b", NL)
    _ts_int(V, ep, tag16, 3, Alu.is_equal)
    ev = S("f16c", NL)
    _ts_int(V, ev, tag16, 1, Alu.is_ge)
    qf = S("f16d", NL)
    _ts_int(V, qf, tag16, 0, Alu.is_equal)

    # ---------- counts ----------
    Cm = S("f16e", NL)
    V.tensor_tensor_scan(Cm, mask16, em, 0.0, Alu.mult, Alu.add)
    C = S("f16c", NL)                  # ev dead after this scan
    V.tensor_tensor_scan(C, mask16, ev, 0.0, Alu.mult, Alu.add)

    # ---------- masked ordinals -> int16 scatter indices ----------
    tm = S("f16f", NL)
    idxm = S("idxm", NL)
    V.tensor_tensor(tm, Cm, em, Alu.mult)
    V.tensor_scalar(idxm, tm, 1.0, None, Alu.subtract)
    pos_m = S("pos_m", NBLK * 64)      # doubled block-local slot of j-th em
    pos_p = S("pos_p", NBLK * 64)
    for b in range(NBLK):
        G.local_scatter(pos_m[:, b * 64:(b + 1) * 64],
                        iotaG2[:, b * LW:(b + 1) * LW],
                        idxm[:, b * LW:(b + 1) * LW], channels=P,
                        num_elems=64, num_idxs=LW)
    cep = S("f16a", NL)                # em dead after tm
    V.tensor_tensor(cep, C, Cm, Alu.subtract)
    V.tensor_tensor(tm, cep, ep, Alu.mult)
    V.tensor_scalar(idxm, tm, 1.0, None, Alu.subtract)
    for b in range(NBLK):
        G.local_scatter(pos_p[:, b * 64:(b + 1) * 64],
                        iotaG2[:, b * LW:(b + 1) * LW],
                        idxm[:, b * LW:(b + 1) * LW], channels=P,
                        num_elems=64, num_idxs=LW)
    tmq = S("f16f", NL)                # tmq = q_ordinal + 1 at query slots
    V.tensor_tensor(tmq, iotaP1G2, C, Alu.subtract)
    V.tensor_tensor(tmq, tmq, qf, Alu.mult)
    idxqL = S("idxm", NL)              # local q ordinal (qpos scatter)
    V.tensor_scalar(idxqL, tmq, 1.0, None, Alu.subtract)
    qpos = S("qpos", NQ)               # doubled block-local slot of k-th query
    for b in range(NBLK):
        G.local_scatter(qpos[:, b * QWS:(b + 1) * QWS],
                        iotaG2[:, b * LW:(b + 1) * LW],
                        idxqL[:, b * LW:(b + 1) * LW], channels=P,
                        num_elems=QWS, num_idxs=LW)
    idx2q = S("idx2q", 2 * NL)         # interleaved (2q, 2q+1)
    i2q3 = idx2q.rearrange("p (n two) -> p n two", two=2)
    G.tensor_scalar(i2q3[:, :, 0], tmq, 2.0, 2.0, Alu.mult, Alu.subtract)
    G.tensor_scalar(i2q3[:, :, 1], tmq, 2.0, 1.0, Alu.mult, Alu.subtract)

    # ---------- radio scatter (fp32 via interleaved doubled idx) ----------
    tgt2 = S("tgt2", NBLK * 256)       # interleaved (2t, 2t+1) event targets
    t2v = tgt2.rearrange("p (b n two) -> p b n two", b=NBLK, two=2)
    G.memset(t2v[:, :, 98:128, :], -1)
    pm3 = _blk(pos_m, 64)
    pp3 = _blk(pos_p, 64)
    G.tensor_copy(t2v[:, :, 0:49, 0], pm3[:, :, 0:49])
    _ts_int(G, t2v[:, :, 0:49, 1], pm3[:, :, 0:49], 1, Alu.add)
    G.tensor_copy(t2v[:, :, 49:98, 0], pp3[:, :, 0:49])
    _ts_int(G, t2v[:, :, 49:98, 1], pp3[:, :, 0:49], 1, Alu.add)

    radcat = S("radcat", NBLK * 128)
    r3 = _blk(radcat, 128)
    G.memset(r3[:, :, 98:128], 0.0)
    G.tensor_copy(r3[:, :, 0:49], _blk(radio[:], 49))
    G.tensor_scalar(r3[:, :, 49:98], _blk(radio[:], 49), -1.0, None, Alu.mult)
    radio_m = S("F_A", NL)
    for b in range(NBLK):
        G.local_scatter(u16v(radio_m, 2 * NL)[:, b * 2 * LW:(b + 1) * 2 * LW],
                        u16v(radcat, 2 * NBLK * 128)[:, b * 256:(b + 1) * 256],
                        tgt2[:, b * 256:(b + 1) * 256], channels=P,
                        num_elems=2 * LW, num_idxs=256)

    # ---------- slope scan ----------
    g = S("F_B", NL)
    V.tensor_tensor_scan(g, mask32, radio_m, 0.0, Alu.mult, Alu.add)

    # ---------- merged exact values via combined scatter ----------
    dat = S("dat", NL)                 # [emsh | epsh | x | pad] per block
    d3 = blkL(dat)
    G.memset(d3[:, :, 98 + X:LW], 0.0)
    nc.scalar.activation(d3[:, :, 0:49], _blk(emsh, 49), AF.Copy)
    nc.scalar.activation(d3[:, :, 49:98], _blk(epsh, 49), AF.Copy)
    nc.scalar.activation(d3[:, :, 98:98 + X], _blk(x_t[:], X), AF.Copy)
    vidx2 = S("vidx2", 2 * NL)
    vi4 = vidx2.rearrange("p (b n two) -> p b n two", b=NBLK, two=2)
    G.memset(vi4[:, :, 98 + X:LW, :], -1)
    G.tensor_copy(vidx2.bitcast(dt.int32).rearrange(
        "p (b n) -> p b n", b=NBLK)[:, :, 0:98],
        tgt2.bitcast(dt.int32).rearrange("p (b n) -> p b n", b=NBLK)[:, :, 0:98])
    q3 = _blk(qpos, QWS)
    G.tensor_copy(vi4[:, :, 98:98 + X, 0], q3[:, :, 0:X])
    _ts_int(G, vi4[:, :, 98:98 + X, 1], q3[:, :, 0:X], 1, Alu.add)
    v = S("F_C", NL)
    for b in range(NBLK):
        G.local_scatter(u16v(v, 2 * NL)[:, b * 2 * LW:(b + 1) * 2 * LW],
                        u16v(dat, 2 * NL)[:, b * 2 * LW:(b + 1) * 2 * LW],
                        vidx2[:, b * 2 * LW:(b + 1) * 2 * LW], channels=P,
                        num_elems=2 * LW, num_idxs=2 * LW)

    # ---------- density reconstruction ----------
    v3 = blkL(v)
    dv = S("F_D", NL)
    dv3 = blkL(dv)
    G.memset(dv3[:, :, 0:1], 0.0)
    G.tensor_tensor(dv3[:, :, 1:LW], v3[:, :, 1:LW], v3[:, :, 0:LW - 1],
                    Alu.subtract)
    wg = S("F_A", NL)                  # radio_m dead after g scan
    wg3 = blkL(wg)
    G.memset(wg3[:, :, 0:1], 0.0)
    G.tensor_tensor(wg3[:, :, 1:LW], dv3[:, :, 1:LW],
                    blkL(g)[:, :, 0:LW - 1], Alu.mult)
    w = S("F_C", NL)                   # v dead after dv
    V.tensor_tensor_scan(w, mask32, wg, 0.0, Alu.mult, Alu.add)
    wc = S("F_A", NL)                  # wg dead after w scan
    nc.scalar.activation(wc, w, AF.Relu, scale=0.5)
    scr = S("F_B", NL)                 # g dead after wg
    scr3 = blkL(scr)
    wc3 = blkL(wc)
    G.memset(scr3[:, :, 0:1], 0.0)
    V.tensor_tensor(scr3[:, :, 1:LW], wc3[:, :, 1:LW], wc3[:, :, 0:LW - 1],
                    Alu.add)
    area = S("F_C", NL)                # w dead after wc
    V.tensor_tensor(area, scr, dv, Alu.mult)
    cdf = S("F_D", NL)                 # dv dead after area
    V.tensor_tensor_scan(cdf, mask32, area, 0.0, Alu.mult, Alu.add)

    # ---------- compact cdf at query slots (fp32 direct) ----------
    cqf = sl["kq32"][:].bitcast(dt.float32)[:, 0:NQ]
    for b in range(NBLK):
        G.local_scatter(u16v(cqf, 2 * NQ)[:, b * 2 * QWS:(b + 1) * 2 * QWS],
                        u16v(cdf, 2 * NL)[:, b * 2 * LW:(b + 1) * 2 * LW],
                        idx2q[:, b * 2 * LW:(b + 1) * 2 * LW], channels=P,
                        num_elems=2 * QWS, num_idxs=2 * LW)

    # ---------- loss tail ----------
    cqf3 = _blk(cqf, QWS)
    ws = S("F_A", NW)                  # wc dead after scr
    V.tensor_tensor(_blk(ws, X - 1), cqf3[:, :, 1:X], cqf3[:, :, 0:X - 1],
                    Alu.subtract)
    V.tensor_tensor(ws, ws, pwt[:], Alu.subtract)
    rsl = S("F_B", NW)                 # scr dead after area
    nc.scalar.activation(rsl, ws, AF.Relu)
    G.tensor_tensor(rsl, rsl, dinv, Alu.mult)
    scrt = S("F_C", NW)
    G.tensor_tensor(scrt, ws, rsl, Alu.mult)
    V.tensor_reduce(inter_acc[:], _blk(scrt, X - 1), AX.XY, Alu.add)
    V.tensor_scalar(inter_acc[:], inter_acc[:], 1.0 / (R * (X - 1)), None,
                    Alu.mult)


def build_module():
    nc = bacc.Bacc("TRN2", target_bir_lowering=False, debug=False,
                   enable_asserts=False, num_devices=N_CORES)
    aps = {}

    def din(name, shape, dtype=dt.float32):
        aps[name] = nc.dram_tensor(name, shape, dtype, kind="ExternalInput").ap()
    din("pd", [RPC, 3]); din("gt", [RPC, 3])
    din("sd", [RPC, 49]); din("rw", [RPC, 48])
    din("ps0", [RPC, 257]); din("pw0", [RPC, 256])
    din("ps1", [RPC, 97]); din("pw1", [RPC, 96])
    din("hi0", [HSLICE], dt.int32); din("he0", [HSLICE * 2])
    din("hi1", [HSLICE], dt.int32); din("he1", [HSLICE * 2])
    out_ap = nc.dram_tensor("out", [1, 1], dt.float32, kind="ExternalOutput").ap()

    with tile.TileContext(nc) as tc:
        _emit(nc, tc, aps, out_ap)
    nc.compile()
    return nc


def _emit(nc, tc, aps, out_ap):
    import contextlib
    V = nc.vector
    G = nc.gpsimd
    with contextlib.ExitStack() as ctx:
        # ---------- generated constants ----------
        cpool = ctx.enter_context(tc.tile_pool(name="consts", bufs=1))
        cst = {}
        accs = {}
        for name in ("rgb", "inter", "inter1", "p1", "p2", "hash"):
            a = cpool.tile([P, 1], dt.float32, tag=f"acc_{name}")
            accs[name] = a
            V.memset(a[:], 0.0)

        # ---------- inputs ----------
        ipool = ctx.enter_context(tc.tile_pool(name="inputs", bufs=1))
        s_sh = ipool.tile([P, NBLK * 49], dt.float32, tag="s_sh")
        nc.sync.dma_start(_blk(s_sh[:], 49),
                          aps["sd"].rearrange("(b p) x -> p b x", p=P))
        rw_sh = ipool.tile([P, NBLK * 48], dt.float32, tag="rw_sh")
        nc.sync.dma_start(_blk(rw_sh[:], 48),
                          aps["rw"].rearrange("(b p) x -> p b x", p=P))
        xts, pwts = {}, {}
        for lvl, L in LVL.items():
            X = L["X"]
            xt = ipool.tile([P, NBLK * X], dt.float32, tag=f"xt{lvl}")
            nc.sync.dma_start(_blk(xt[:], X),
                              aps[f"ps{lvl}"].rearrange("(b p) x -> p b x", p=P))
            xts[lvl] = xt
            pwt = ipool.tile([P, NBLK * (X - 1)], dt.float32, tag=f"pwt{lvl}")
            nc.scalar.dma_start(_blk(pwt[:], X - 1),
                                aps[f"pw{lvl}"].rearrange("(b p) x -> p b x", p=P))
            pwts[lvl] = pwt
        pdt = ipool.tile([P, NBLK * 3], dt.float32, tag="pdt")
        gtt = ipool.tile([P, NBLK * 3], dt.float32, tag="gtt")
        nc.sync.dma_start(_blk(pdt[:], 3),
                          aps["pd"].rearrange("(b p) c -> p b c", p=P))
        nc.sync.dma_start(_blk(gtt[:], 3),
                          aps["gt"].rearrange("(b p) c -> p b c", p=P))
        hidx, hemb = {}, {}
        for lvl in (0, 1):
            hi_t = ipool.tile([P, HCOLS], dt.int32, tag=f"hidx{lvl}")
            src = aps[f"hi{lvl}"]
            nc.sync.dma_start(hi_t[:], bass.AP(tensor=src.tensor,
                                               offset=src.offset,
                                               ap=[[HROW, P], [1, HCOLS]]))
            hidx[lvl] = hi_t
            he_t = ipool.tile([P, HCOLS * 2], dt.float32, tag=f"hemb{lvl}")
            esrc = aps[f"he{lvl}"]
            nc.scalar.dma_start(he_t[:], bass.AP(tensor=esrc.tensor,
                                                 offset=esrc.offset,
                                                 ap=[[HROW * 2, P],
                                                     [1, HCOLS * 2]]))
            hemb[lvl] = he_t

        # iotaG consts (needed first by mid-level scatters)
        for lvl, L in LVL.items():
            LW = L["LW"]
            NL = NBLK * LW
            t = cpool.tile([P, NL], dt.int16, tag=f"iotaG2_l{lvl}",
                           name=f"iotaG2_l{lvl}")
            G.iota(t[:], [[0, NBLK], [2, LW]], base=0, channel_multiplier=0)
            cst[f"iotaG2_l{lvl}"] = t

        # ---------- shared prep: wnorm, radio, mrad (Pool) ----------
        spool = ctx.enter_context(tc.tile_pool(name="shared", bufs=1))
        s3 = _blk(s_sh[:], 49)
        ds = spool.tile([P, NBLK * 48], dt.float32, tag="ds")
        G.tensor_tensor(_blk(ds[:], 48), s3[:, :, 1:49], s3[:, :, 0:48],
                        Alu.subtract)
        dse = spool.tile([P, NBLK * 48], dt.float32, tag="dse")
        G.tensor_scalar(dse[:], ds[:], 1e-8, None, Alu.add)
        wnorm = spool.tile([P, NBLK * 48], dt.float32, tag="wnorm")
        V.reciprocal(dse[:], dse[:])
        G.tensor_tensor(wnorm[:], rw_sh[:], dse[:], Alu.mult)
        wnp = spool.tile([P, NBLK * 50], dt.float32, tag="wnp")
        G.memset(wnp[:], 0.0)
        G.tensor_copy(_blk(wnp[:], 50)[:, :, 1:49], _blk(wnorm[:], 48))
        diff = spool.tile([P, NBLK * 49], dt.float32, tag="diff")
        wnp3 = _blk(wnp[:], 50)
        G.tensor_tensor(_blk(diff[:], 49), wnp3[:, :, 1:50], wnp3[:, :, 0:49],
                        Alu.subtract)
        radios = {}
        for lvl in (0, 1):
            rt = spool.tile([P, NBLK * 49], dt.float32, tag=f"radio{lvl}")
            G.tensor_scalar(rt[:], diff[:], 1.0 / (2 * PULSE[lvl]), None,
                            Alu.mult)
            radios[lvl] = rt

        # remaining consts: masks + iotaP1G (needed mid-level)
        iP1i = cpool.tile([P, NL0], dt.int16, tag="iP1i")
        for lvl, L in LVL.items():
            LW, QWS, NL = L["LW"], L["QWS"], NBLK * L["LW"]
            m16 = cpool.tile([P, NL], dt.float16, tag=f"mask16_l{lvl}",
                             name=f"mask16_l{lvl}")
            G.memset(m16[:], 1.0)
            G.memset(_blk(m16[:], LW)[:, :, 0:1], 0.0)
            cst[f"mask16_l{lvl}"] = m16
            m32 = cpool.tile([P, NL], dt.float32, tag=f"mask32_l{lvl}",
                             name=f"mask32_l{lvl}")
            G.memset(m32[:], 1.0)
            G.memset(_blk(m32[:], LW)[:, :, 0:1], 0.0)
            cst[f"mask32_l{lvl}"] = m32
            G.iota(iP1i[:][:, 0:NL], [[0, NBLK], [1, LW]], base=1,
                   channel_multiplier=0)
            tf = cpool.tile([P, NL], dt.float16, tag=f"iotaP1G2_l{lvl}",
                            name=f"iotaP1G2_l{lvl}")
            nc.scalar.activation(tf[:], iP1i[:][:, 0:NL], AF.Copy)
            cst[f"iotaP1G2_l{lvl}"] = tf
        mask48 = cpool.tile([P, NBLK * 48], dt.float32, tag="mask48")
        G.memset(mask48[:], 1.0)
        G.memset(_blk(mask48[:], 48)[:, :, 0:1], 0.0)
        ones_h = cpool.tile([P, HCOLS], dt.float32, tag="ones_h")
        G.memset(ones_h[:], 1.0)

        # ---------- per-level slabs ----------
        lpool = ctx.enter_context(tc.tile_pool(name="levels", bufs=1))
        slabs = {0: {}, 1: {}}
        for lvl, L in LVL.items():
            sl = slabs[lvl]
            NL = NBLK * L["LW"]
            NQ = NBLK * L["QWS"]
            n2 = L["n2"]

            def slab(name, width, dtype, lvl=lvl, sl=sl):
                nm = f"{name}_l{lvl}"
                sl[name] = lpool.tile([P, width], dtype, tag=nm, name=nm)
            slab("emsh", NBLK * 49, dt.float32)
            slab("epsh", NBLK * 49, dt.float32)
            slab("k32", NBLK * 49, dt.int32)
            slab("kq32", NQ, dt.int32)
            slab("b1a", NBLK * 128, dt.uint16)
            slab("b1b", NBLK * 128, dt.uint16)
            slab("B0", NBLK * n2, dt.uint16)
            slab("B1", NBLK * n2, dt.uint16)
            slab("tag16", NL, dt.uint16)
            for nm in ("f16a", "f16b", "f16c", "f16d", "f16e", "f16f"):
                slab(nm, NL, dt.float16)
            slab("idxm", NL, dt.int16)
            slab("idx2q", 2 * NL, dt.int16)
            slab("vidx2", 2 * NL, dt.int16)
            slab("pos_m", NBLK * 64, dt.int16)
            slab("pos_p", NBLK * 64, dt.int16)
            slab("qpos", NQ, dt.int16)
            slab("tgt2", NBLK * 256, dt.int16)
            slab("radcat", NBLK * 128, dt.float32)
            slab("dat", NL, dt.float32)
            for nm in ("F_A", "F_B", "F_C", "F_D"):
                slab(nm, NL, dt.float32)
            slab("dinv", NBLK * (L["X"] - 1), dt.float32)

        for lvl in (0, 1):
            _emit_level(nc, tc, slabs[lvl], lvl, s_sh, radios[lvl],
                        xts[lvl], pwts[lvl],
                        accs["inter" if lvl == 0 else "inter1"], cst)

        # ---------- distortion loss (Pool) ----------
        with tc.tile_pool(name="dist", bufs=1) as pool:
            mid = pool.tile([P, NBLK * 48], dt.float32, tag="mid")
            G.tensor_tensor(_blk(mid[:], 48), s3[:, :, 1:49],
                            s3[:, :, 0:48], Alu.add)
            G.tensor_scalar(mid[:], mid[:], 0.5, None, Alu.mult)
            wm = pool.tile([P, NBLK * 48], dt.float32, tag="wm")
            G.tensor_tensor(wm[:], rw_sh[:], mid[:], Alu.mult)
            Cin = pool.tile([P, NBLK * 48], dt.float32, tag="Cin")
            V.tensor_tensor_scan(Cin[:], mask48[:], rw_sh[:], 0.0,
                                 Alu.mult, Alu.add)
            Sin = pool.tile([P, NBLK * 48], dt.float32, tag="Sin")
            V.tensor_tensor_scan(Sin[:], mask48[:], wm[:], 0.0,
                                 Alu.mult, Alu.add)
            A = pool.tile([P, NBLK * 47], dt.float32, tag="A47")
            m3 = _blk(mid[:], 48)
            c3 = _blk(Cin[:], 48)
            sw3 = _blk(Sin[:], 48)
            rw3 = _blk(rw_sh[:], 48)
            A3 = _blk(A[:], 47)
            G.tensor_tensor(A3, m3[:, :, 1:48], c3[:, :, 0:47], Alu.mult)
            G.tensor_tensor(A3, A3, sw3[:, :, 0:47], Alu.subtract)
            G.tensor_tensor(A3, A3, rw3[:, :, 1:48], Alu.mult)
            V.tensor_reduce(accs["p1"][:], A3, AX.XY, Alu.add)
            t2 = pool.tile([P, NBLK * 48], dt.float32, tag="t2d")
            G.tensor_tensor(t2[:], rw_sh[:], rw_sh[:], Alu.mult)
            G.tensor_tensor(t2[:], t2[:], ds[:], Alu.mult)
            V.tensor_reduce(accs["p2"][:], _blk(t2[:], 48), AX.XY, Alu.add)

        # ---------- rgb (Pool) ----------
        with tc.tile_pool(name="rgb", bufs=1) as pool:
            d = pool.tile([P, NBLK * 3], dt.float32, tag="rgbd")
            G.tensor_tensor(d[:], pdt[:], gtt[:], Alu.subtract)
            G.tensor_tensor(d[:], d[:], d[:], Alu.mult)
            V.tensor_reduce(accs["rgb"][:], d[:], AX.X, Alu.add)

        # ---------- hash (Pool) ----------
        for lvl in (0, 1):
            with tc.tile_pool(name=f"hash{lvl}", bufs=1) as pool:
                idx = hidx[lvl]
                emb = hemb[lvl]
                sq = pool.tile([P, HCOLS * 2], dt.float32, tag="hsq")
                G.tensor_tensor(sq[:], emb[:], emb[:], Alu.mult)
                wv = pool.tile([P, HCOLS], dt.float32, tag="hw")
                sq3 = sq[:].rearrange("p (n two) -> p n two", two=2)
                G.tensor_tensor(wv[:], sq3[:, :, 0], sq3[:, :, 1], Alu.add)
                eq = pool.tile([P, HCOLS], dt.float32, tag="heq")
                G.memset(eq[:, 0:1], 0.0)
                dq = pool.tile([P, HCOLS], dt.int32, tag="hdq")
                G.tensor_tensor(dq[:, 1:HCOLS], idx[:, 1:HCOLS],
                                idx[:, 0:HCOLS - 1], Alu.subtract)
                _ts_int(G, eq[:, 1:HCOLS], dq[:, 1:HCOLS], 0, Alu.is_equal)
                S = pool.tile([P, HCOLS], dt.float32, tag="hS")
                V.tensor_tensor_scan(S[:], eq[:], wv[:], 0.0,
                                     Alu.mult, Alu.add)
                cc = pool.tile([P, HCOLS], dt.float32, tag="hcc")
                V.tensor_tensor_scan(cc[:], eq[:], ones_h[:], 0.0,
                                     Alu.mult, Alu.add)
                ratio = pool.tile([P, HCOLS], dt.float32, tag="hr")
                V.reciprocal(cc[:], cc[:])
                G.tensor_tensor(ratio[:], S[:], cc[:], Alu.mult)
                me = pool.tile([P, HCOLS], dt.float32, tag="hme")
                G.tensor_scalar(me[:, 0:HCOLS - 1], eq[:, 1:HCOLS], -1.0,
                                1.0, Alu.mult, Alu.add)
                G.tensor_tensor(ratio[:, HALO:HALO + HROW],
                                ratio[:, HALO:HALO + HROW],
                                me[:, HALO:HALO + HROW], Alu.mult)
                part = pool.tile([P, 1], dt.float32, tag="hpart")
                V.tensor_reduce(part[:], ratio[:, HALO:HALO + HROW],
                                AX.X, Alu.add)
                if lvl == 0:
                    V.tensor_copy(accs["hash"][:], part[:])
                else:
                    V.tensor_tensor(accs["hash"][:], accs["hash"][:],
                                    part[:], Alu.add)

        # ---------- combine + output ----------
        with tc.tile_pool(name="fin", bufs=1) as pool:
            tot = pool.tile([P, 1], dt.float32, tag="tot")
            V.tensor_scalar(tot[:], accs["rgb"][:], W_RGB / (R * 3), None,
                            Alu.mult)
            V.scalar_tensor_tensor(tot[:], accs["inter"][:], W_INTER,
                                   tot[:], Alu.mult, Alu.add)
            V.scalar_tensor_tensor(tot[:], accs["inter1"][:], W_INTER,
                                   tot[:], Alu.mult, Alu.add)
            V.scalar_tensor_tensor(tot[:], accs["p1"][:], W_DIST * 2.0 / R,
                                   tot[:], Alu.mult, Alu.add)
            V.scalar_tensor_tensor(tot[:], accs["p2"][:],
                                   W_DIST / (3.0 * R), tot[:],
                                   Alu.mult, Alu.add)
            V.scalar_tensor_tensor(tot[:], accs["hash"][:],
                                   W_HASH / (NUM_SEGMENTS * 2.0), tot[:],
                                   Alu.mult, Alu.add)
            res = pool.tile([P, 1], dt.float32, tag="res")
            G.partition_all_reduce(res[:], tot[:], channels=P,
                                   reduce_op=bass_isa.ReduceOp.add)
            nc.sync.dma_start(out_ap, res[0:1, :])


# ---------------- host side ----------------
_module_cache = {}


def _get_module():
    if "nc" not in _module_cache:
        _module_cache["nc"] = build_module()
    return _module_cache["nc"]


def shard_inputs(inputs):
    """Full inputs -> list of 8 per-core in_maps."""
    f32 = np.float32
    pd = np.ascontiguousarray(inputs["pd_rgbs"], f32)
    gt = np.ascontiguousarray(inputs["gt_rgbs"], f32)
    sd = np.ascontiguousarray(inputs["render_sdist"], f32)
    rw = np.ascontiguousarray(inputs["render_weights"], f32)
    ps0 = np.ascontiguousarray(inputs["prop_sdist_0"], f32)
    pw0 = np.ascontiguousarray(inputs["prop_weights_0"], f32)
    ps1 = np.ascontiguousarray(inputs["prop_sdist_1"], f32)
    pw1 = np.ascontiguousarray(inputs["prop_weights_1"], f32)
    hashes = {}
    for lvl in (0, 1):
        idx = np.asarray(inputs[f"enc_idx_{lvl}"]).astype(np.int32)
        emb = np.ascontiguousarray(inputs[f"enc_embds_{lvl}"], f32)
        idx_pad = np.full(M + 2 * HALO, -1, np.int32)
        idx_pad[HALO:HALO + M] = idx
        emb_pad = np.zeros((M + 2 * HALO, 2), f32)
        emb_pad[HALO:HALO + M] = emb
        hashes[lvl] = (idx_pad, emb_pad)

    in_maps = []
    for c in range(N_CORES):
        r0 = c * RPC
        lo = c * MPC
        im = {
            "pd": pd[r0:r0 + RPC], "gt": gt[r0:r0 + RPC],
            "sd": sd[r0:r0 + RPC], "rw": rw[r0:r0 + RPC],
            "ps0": ps0[r0:r0 + RPC], "pw0": pw0[r0:r0 + RPC],
            "ps1": ps1[r0:r0 + RPC], "pw1": pw1[r0:r0 + RPC],
        }
        for lvl in (0, 1):
            idx_pad, emb_pad = hashes[lvl]
            im[f"hi{lvl}"] = np.ascontiguousarray(idx_pad[lo:lo + HSLICE])
            im[f"he{lvl}"] = np.ascontiguousarray(
                emb_pad[lo:lo + HSLICE].reshape(-1))
        in_maps.append(im)
    return in_maps


def kernel(**inputs) -> np.ndarray:
    nc = _get_module()
    in_maps = shard_inputs(inputs)
    res = run_bass_kernel_spmd(nc, in_maps, core_ids=list(range(N_CORES)))
    total = np.float64(0.0)
    for r in res.results:
        total += np.float64(r["out"][0, 0])
    return np.float32(total)
